# revision 1
# baseline (speedup 1.0000x reference)
"""TRN2 Bass kernel for nn_CardClassifier: CNN(4x conv3x3+relu+maxpool2) ->
per-feature sigmoid attention -> 128 stacked expert MLPs -> fusion MLP.

Sharding: pure data parallel. 8 cores x 4 images, weights replicated.
Single kernel launch per core, no collectives. Expert/fusion weights are
streamed from HBM as bf16 stationary operands (FWL); convs run in float32r.
"""

import sys

sys.path.insert(0, "/opt/trn_rl_repo")

import json as _json
import numpy as np
import ml_dtypes

import concourse.bass as bass
import concourse.mybir as mybir
from concourse import tile
from concourse.bass_utils import run_bass_kernel_spmd

F32 = mybir.dt.float32
F32R = mybir.dt.float32r
BF16 = mybir.dt.bfloat16
AF = mybir.ActivationFunctionType
ALU = None  # filled lazily

B, CIN, H, W = 32, 3, 224, 224
NCORES, BL = 8, 4  # 4 images per core
CHANS = [3, 32, 64, 128, 128]
NF, FLAT = 128, 196
EXP_DIMS = [196, 196, 196, 98, 24, 16]
FIN_DIMS = [128 * 16, 2038, 2028, 53]

# dtype knobs
EW_NP = ml_dtypes.bfloat16   # expert weights + activations
FW_NP = ml_dtypes.bfloat16   # fusion weights + activations
EW_DT, FW_DT = BF16, BF16

_BUILT = None  # cached nc
DEBUG = False


# ---------------------------------------------------------------- tilefix
def _fix_bir_json(raw: bytes) -> bytes:
    """This walrus build allows at most 1 sync-wait per instruction; Tile's
    tail drain can carry more. Split extras onto NoOp carriers (same engine,
    inserted just before, so stream order semantics are unchanged)."""
    d = _json.loads(raw)
    k = 0
    for fn in d.get("functions", []):
        for blk in fn.get("blocks", []):
            out = []
            for inst in blk["instructions"]:
                si = inst.get("sync_info")
                waits = (si or {}).get("on_wait") or []
                if len(waits) > 1:
                    for wchunk in waits[:-1]:
                        out.append({
                            "debug": inst.get("debug", 0),
                            "engine": inst["engine"],
                            "ins": [], "outs": [],
                            "name": f"NOPW-{k}",
                            "opcode": "NoOp",
                            "sync_info": {"on_update": [], "on_wait": [wchunk]},
                        })
                        k += 1
                    si["on_wait"] = waits[-1:]
                out.append(inst)
            blk["instructions"] = out
    return _json.dumps(d).encode()


# ---------------------------------------------------------------- build
def _build():
    global ALU
    from concourse.alu_op_type import AluOpType as ALU_

    ALU = ALU_
    nc = bass.Bass("TRN2", target_bir_lowering=False, debug=False)

    dp = lambda name, shape, dt: nc.declare_dram_parameter(name, list(shape), dt, isOutput=False)

    x_in = dp("x", [BL, 3, H, W], F32R)
    ident_in = dp("ident", [128, 128], F32R)
    cw_in = [dp(f"cwr{i}", [128, 9, 128], F32R) for i in range(4)]
    cb_in = [dp(f"cbr{i}", [128, 1], F32) for i in range(4)]
    aw_in = dp("awr", [128, FLAT], F32)
    ab_in = dp("abr", [128, 1], F32)

    # expert weights, K-major chunked [K, 128e, ochunk], bias appended to B rows
    ew_shapes = {
        "e1AA": (128, 128, 128), "e1AB": (128, 128, 68), "e1BA": (69, 128, 128), "e1BB": (69, 128, 68),
        "e2AA": (128, 128, 128), "e2AB": (128, 128, 68), "e2BA": (69, 128, 128), "e2BB": (69, 128, 68),
        "e3A": (128, 128, 98), "e3B": (69, 128, 98),
        "e4": (99, 128, 24),
        "e5": (25, 128, 16),
    }
    ew_in = {k: dp(k, list(s), EW_DT) for k, s in ew_shapes.items()}

    # fusion weights: fw1 rows permuted on host to match F0 layout
    fw1_in = dp("fw1p", [16, 128, FIN_DIMS[1]], FW_DT)   # [slice, K=128, 2038]
    fb1_in = dp("fb1r", [1, FIN_DIMS[1]], FW_DT)
    fw2_in = dp("fw2r", [FIN_DIMS[1], FIN_DIMS[2]], FW_DT)
    fb2_in = dp("fb2r", [1, FIN_DIMS[2]], FW_DT)
    fw3_in = dp("fw3r", [FIN_DIMS[2], FIN_DIMS[3]], FW_DT)
    fb3_in = dp("fb3r", [1, FIN_DIMS[3]], FW_DT)
    ones_in = dp("onesrow", [1, 512], EW_DT)

    y_out = nc.declare_dram_parameter("y", [BL, 53], F32, isOutput=True)
    if DEBUG:
        dbg = {
            "dbg_feats": nc.declare_dram_parameter("dbg_feats", [BL, 128, FLAT], F32, isOutput=True),
            "dbg_ta": nc.declare_dram_parameter("dbg_ta", [128, 512], EW_DT, isOutput=True),
            "dbg_tb": nc.declare_dram_parameter("dbg_tb", [69, 512], EW_DT, isOutput=True),
            "dbg_t2a": nc.declare_dram_parameter("dbg_t2a", [128, 512], EW_DT, isOutput=True),
            "dbg_t3a": nc.declare_dram_parameter("dbg_t3a", [128, 512], EW_DT, isOutput=True),
            "dbg_o5": nc.declare_dram_parameter("dbg_o5", [16, 512], FW_DT, isOutput=True),
            "dbg_fsb": nc.declare_dram_parameter("dbg_fsb", [128, 64], FW_DT, isOutput=True),
            "dbg_s1": nc.declare_dram_parameter("dbg_s1", [128, 64], F32, isOutput=True),
            "dbg_s2": nc.declare_dram_parameter("dbg_s2", [128, 64], F32, isOutput=True),
        }

    r32 = lambda ap: ap.bitcast(F32R)

    with tile.TileContext(nc, pool_alloc_mode="queue") as tc:
        import contextlib

        stk = contextlib.ExitStack()
        with stk:
            # ---- persistent pools
            wpool = stk.enter_context(tc.tile_pool(name="wconst", bufs=1))
            cw = []
            for i in range(4):
                t = wpool.tile(list(ew_shapes_cw(i)), F32R, name=f"cwsb{i}")
                nc.gpsimd.dma_start(t[:], cw_in[i][:])
                cw.append(t)
            cb = []
            for i in range(4):
                t = wpool.tile([128, 1], F32, name=f"cbsb{i}")
                nc.gpsimd.dma_start(t[:], cb_in[i][:])
                cb.append(t)
            awsb = wpool.tile([128, FLAT], F32)
            nc.gpsimd.dma_start(awsb[:], aw_in[:])
            absb = wpool.tile([128, 1], F32)
            nc.gpsimd.dma_start(absb[:], ab_in[:])
            ident = wpool.tile([128, 128], F32R)
            nc.gpsimd.dma_start(ident[:], ident_in[:])

            featpool = stk.enter_context(tc.tile_pool(name="feats", bufs=1))
            feats = [featpool.tile([128, FLAT], F32R, name=f"feats{i}") for i in range(BL)]
            tpool = stk.enter_context(tc.tile_pool(name="texp", bufs=1))
            epool = stk.enter_context(tc.tile_pool(name="ew", bufs=4))
            fpool = stk.enter_context(tc.tile_pool(name="fw", bufs=2))

            psum_c = stk.enter_context(tc.tile_pool(name="psacc", bufs=5, space="PSUM"))
            psum_t = stk.enter_context(tc.tile_pool(name="pstr", bufs=2, space="PSUM"))

            import contextlib as _ctx
            actstk = _ctx.ExitStack()
            apool = actstk.enter_context(tc.tile_pool(name="acts", bufs=1))

            # =========================================================
            # conv1: 3->32, 224x224, K=3 per img, 4 imgs diagonal-packed
            # strips of 14 output rows (16 strips)
            # =========================================================
            A1p = apool.tile([128, 114 * 114], F32R)
            SR = 8
            NS = H // SR
            a1v = A1p.rearrange("p (r c) -> p r c", c=114)
            nc.vector.memset(a1v[:, 0, :].bitcast(F32), 0.0)
            nc.vector.memset(a1v[:, 113, :].bitcast(F32), 0.0)
            nc.vector.memset(a1v[:, :, 0].bitcast(F32), 0.0)
            nc.vector.memset(a1v[:, :, 113].bitcast(F32), 0.0)

            with tc.tile_pool(name="c1", bufs=2) as c1pool, \
                 tc.tile_pool(name="c1o", bufs=2) as c1opool, \
                 tc.tile_pool(name="c1v", bufs=2) as c1vpool:
                for s in range(NS):
                    r0 = s * SR
                    xs = c1pool.tile([128, (SR + 2) * 226], F32R, tag="xs")
                    xv = xs.rearrange("p (r c) -> p r c", c=226)
                    nc.vector.memset(xv[:, :, 0].bitcast(F32), 0.0)
                    nc.vector.memset(xv[:, :, 225].bitcast(F32), 0.0)
                    if s == 0:
                        nc.vector.memset(xv[:, 0, :].bitcast(F32), 0.0)
                    if s == NS - 1:
                        nc.vector.memset(xv[:, SR + 1, :].bitcast(F32), 0.0)
                    lo = max(r0 - 1, 0)
                    hi = min(r0 + SR + 1, H)
                    dro = lo - (r0 - 1)
                    for img in range(BL):
                        nc.gpsimd.dma_start(
                            xv[3 * img:3 * img + 3, dro:dro + (hi - lo), 1:225],
                            x_in[img, :, lo:hi, :],
                        )
                    o1 = c1opool.tile([128, SR * 224], F32R, tag="o1")
                    for t in range(SR // 2):
                        P = psum_c.tile([128, 448], F32, tag="acc")
                        for k in range(9):
                            ky, kx = divmod(k, 3)
                            rhs = xv[0:12, 2 * t + ky:2 * t + ky + 2, kx:kx + 224]
                            nc.tensor.matmul(
                                P[:], cw[0][0:12, k, :], rhs,
                                start=(k == 0), stop=(k == 8))
                        nc.scalar.activation(o1[:, t * 448:(t + 1) * 448], P[:], AF.Relu, bias=cb[0][:])
                    o1v = o1.rearrange("p (r c) -> p r c", c=224)
                    pv = c1vpool.tile([128, (SR // 2) * 224], F32R, tag="pv")
                    pvv = pv.rearrange("p (r c) -> p r c", c=224)
                    nc.vector.tensor_tensor(pvv[:], o1v[:, 0:SR:2, :], o1v[:, 1:SR:2, :], op=ALU.max)
                    nc.vector.tensor_tensor(
                        a1v[:, 1 + s * (SR // 2):1 + (s + 1) * (SR // 2), 1:113],
                        pvv[:, :, 0:224:2], pvv[:, :, 1:224:2], op=ALU.max,
                    )

            # =========================================================
            # conv2: 32->64, 112x112, K=32 per img, img pairs via col groups
            # =========================================================
            A2p = [apool.tile([128, 58 * 58], F32R, name=f"A2p{i}") for i in range(2)]
            for p2 in range(2):
                a2v = A2p[p2].rearrange("p (r c) -> p r c", c=58)
                nc.vector.memset(a2v[:, 0, :].bitcast(F32), 0.0)
                nc.vector.memset(a2v[:, 57, :].bitcast(F32), 0.0)
                nc.vector.memset(a2v[:, :, 0].bitcast(F32), 0.0)
                nc.vector.memset(a2v[:, :, 57].bitcast(F32), 0.0)

            with tc.tile_pool(name="c2o", bufs=2) as c2opool, \
                 tc.tile_pool(name="c2v", bufs=2) as c2vpool:
                for pr in range(2):
                    a2v = A2p[pr].rearrange("p (r c) -> p r c", c=58)
                    a1vv = A1p.rearrange("p (r c) -> p r c", c=114)
                    for th in range(14):   # 8-row chunks = 2 n-tiles of 4 rows
                        o2 = c2opool.tile([128, 8 * 112], F32R, tag="o2")
                        for tt in range(2):
                            t = 2 * th + tt
                            P = psum_c.tile([128, 448], F32, tag="acc")
                            for k in range(9):
                                ky, kx = divmod(k, 3)
                                rhs = a1vv[64 * pr:64 * pr + 64, 4 * t + ky:4 * t + ky + 4, kx:kx + 112]
                                nc.tensor.matmul(
                                    P[:],
                                    cw[1][64 * pr:64 * pr + 64, k, :],
                                    rhs,
                                    start=(k == 0), stop=(k == 8),
                                )
                            nc.scalar.activation(o2[:, tt * 448:(tt + 1) * 448], P[:], AF.Relu, bias=cb[1][:])
                        o2v = o2.rearrange("p (r c) -> p r c", c=112)
                        pv = c2vpool.tile([128, 4 * 112], F32R, tag="pv2")
                        pvv = pv.rearrange("p (r c) -> p r c", c=112)
                        nc.vector.tensor_tensor(pvv[:], o2v[:, 0:8:2, :], o2v[:, 1:8:2, :], op=ALU.max)
                        nc.vector.tensor_tensor(
                            a2v[:, 1 + 4 * th:1 + 4 * (th + 1), 1:57],
                            pvv[:, :, 0:112:2], pvv[:, :, 1:112:2], op=ALU.max)
            # =========================================================
            # conv3: 64->128, 56x56, K=64, serial per img
            # =========================================================
            A3p = [apool.tile([128, 30 * 30], F32R, name=f"A3p{i}") for i in range(BL)]
            for img in range(BL):
                a3v = A3p[img].rearrange("p (r c) -> p r c", c=30)
                nc.vector.memset(a3v[:, 0, :].bitcast(F32), 0.0)
                nc.vector.memset(a3v[:, 29, :].bitcast(F32), 0.0)
                nc.vector.memset(a3v[:, :, 0].bitcast(F32), 0.0)
                nc.vector.memset(a3v[:, :, 29].bitcast(F32), 0.0)

            with tc.tile_pool(name="c3o", bufs=2) as c3opool, \
                 tc.tile_pool(name="c3v", bufs=2) as c3vpool:
                for img in range(BL):
                    pr, sl = divmod(img, 2)
                    a2v = A2p[pr].rearrange("p (r c) -> p r c", c=58)
                    a3v = A3p[img].rearrange("p (r c) -> p r c", c=30)
                    for t in range(7):   # 8 rows x 56 = 448
                        o3 = c3opool.tile([128, 8 * 56], F32R, tag="o3")
                        P = psum_c.tile([128, 448], F32, tag="acc")
                        for k in range(9):
                            ky, kx = divmod(k, 3)
                            rhs = a2v[64 * sl:64 * sl + 64, 8 * t + ky:8 * t + ky + 8, kx:kx + 56]
                            nc.tensor.matmul(
                                P[:], cw[2][64 * sl:64 * sl + 64, k, :], rhs,
                                start=(k == 0), stop=(k == 8),
                            )
                        nc.scalar.activation(o3[:], P[:], AF.Relu, bias=cb[2][:])
                        o3v = o3.rearrange("p (r c) -> p r c", c=56)
                        pv = c3vpool.tile([128, 4 * 56], F32R, tag="pv3")
                        pvv = pv.rearrange("p (r c) -> p r c", c=56)
                        nc.vector.tensor_tensor(pvv[:], o3v[:, 0:8:2, :], o3v[:, 1:8:2, :], op=ALU.max)
                        nc.vector.tensor_tensor(
                            a3v[:, 1 + 4 * t:1 + 4 * (t + 1), 1:29],
                            pvv[:, :, 0:56:2], pvv[:, :, 1:56:2], op=ALU.max)

            # =========================================================
            # conv4: 128->128, 28x28, K=128, serial per img
            # =========================================================
            with tc.tile_pool(name="c4o", bufs=2) as c4opool, \
                 tc.tile_pool(name="c4v", bufs=2) as c4vpool:
                for img in range(BL):
                    a3v = A3p[img].rearrange("p (r c) -> p r c", c=30)
                    fv = feats[img].rearrange("p (r c) -> p r c", c=14)
                    for t in range(2):   # 14 rows x 28 = 392
                        o4 = c4opool.tile([128, 14 * 28], F32R, tag="o4")
                        P = psum_c.tile([128, 392], F32, tag="acc")
                        for k in range(9):
                            ky, kx = divmod(k, 3)
                            rhs = a3v[:, 14 * t + ky:14 * t + ky + 14, kx:kx + 28]
                            nc.tensor.matmul(
                                P[:], cw[3][:, k, :], rhs,
                                start=(k == 0), stop=(k == 8))
                        nc.scalar.activation(o4[:], P[:], AF.Relu, bias=cb[3][:])
                        o4v = o4.rearrange("p (r c) -> p r c", c=28)
                        pv = c4vpool.tile([128, 7 * 28], F32R, tag="pv4")
                        pvv = pv.rearrange("p (r c) -> p r c", c=28)
                        nc.vector.tensor_tensor(pvv[:], o4v[:, 0:14:2, :], o4v[:, 1:14:2, :], op=ALU.max)
                        nc.vector.tensor_tensor(
                            fv[:, 7 * t:7 * (t + 1), :],
                            pvv[:, :, 0:28:2], pvv[:, :, 1:28:2], op=ALU.max)
            actstk.close()

            # =========================================================
            # attention + h = feats*att; build T1 (bf16) via PE transposes
            # =========================================================
            Ta = tpool.tile([128, 512], EW_DT)      # [i<128, e*4+img]
            Tb = tpool.tile([69, 512], EW_DT)

            with tc.tile_pool(name="attp", bufs=2) as attpool:
                for img in range(BL):
                    tmp = attpool.tile([128, FLAT], F32, tag="tmp")
                    nc.vector.tensor_tensor(tmp[:], feats[img][:], awsb[:], op=ALU.mult)
                    attv = attpool.tile([128, 1], F32, tag="attv")
                    nc.vector.tensor_reduce(attv[:], tmp[:], axis=mybir.AxisListType.X, op=ALU.add)
                    atts = attpool.tile([128, 1], F32, tag="atts")
                    nc.scalar.activation(atts[:], attv[:], AF.Sigmoid, bias=absb[:])
                    himg = attpool.tile([128, FLAT], F32R, tag="h")
                    nc.vector.tensor_scalar(himg[:], feats[img][:], atts[:, 0:1], None, op0=ALU.mult)
                    # transpose h -> T1 columns img::4
                    PT = psum_t.tile([128, 128], F32R, tag="tr")
                    nc.tensor.transpose(PT[:], himg[:, 0:128], ident[:])
                    nc.vector.tensor_copy(Ta.rearrange("p (e i) -> p e i", i=4)[:, :, img], PT[:])
                    PT2 = psum_t.tile([128, 128], F32R, tag="tr")
                    nc.tensor.transpose(PT2[0:68, :], himg[:, 128:196], ident[:])
                    nc.vector.tensor_copy(Tb.rearrange("p (e i) -> p e i", i=4)[0:68, :, img], PT2[0:68, :])
            nc.gpsimd.dma_start(Tb[68:69, :], ones_in[:])
            if DEBUG:
                for img in range(BL):
                    nc.gpsimd.dma_start(dbg["dbg_feats"][img], feats[img][:].bitcast(F32))
                nc.gpsimd.dma_start(dbg["dbg_ta"][:], Ta[:])
                nc.gpsimd.dma_start(dbg["dbg_tb"][:], Tb[:])

            # =========================================================
            # experts: 5 layers, stationary=ew (bf16+FWL), moving=T slices
            # outputs land directly in next layer's T layout
            # =========================================================
            G = 16  # experts per weight-DMA group
            psum_e = psum_c

            # L1/L2: in 196 (A128+B69), out 196 (A128 + B68)
            def full_layer(Tin_a, Tin_b, pre, last_relu=True):
                PA = psum_e.tile([128, 512], F32, tag="acc")
                PB = psum_e.tile([68, 512], F32, tag="acc")
                for g in range(128 // G):
                    wts = {}
                    for sfx, (kd, od) in {"AA": (128, 128), "AB": (128, 68), "BA": (69, 128), "BB": (69, 68)}.items():
                        name = pre + sfx
                        t = epool.tile([kd, G * od], EW_DT, tag="ew", name=name + "t")
                        nc.gpsimd.dma_start(
                            t.rearrange("k (e o) -> k e o", o=od)[:],
                            ew_in[name][:, g * G:(g + 1) * G, :])
                        wts[sfx] = t
                    for j in range(G):
                        e = g * G + j
                        sl = slice(4 * e, 4 * e + 4)
                        nc.tensor.matmul(PA[:, sl], wts["AA"].rearrange("k (e o) -> k e o", o=128)[:, j, :],
                                         Tin_a[:, sl], start=True, stop=False)
                        nc.tensor.matmul(PA[:, sl], wts["BA"].rearrange("k (e o) -> k e o", o=128)[:, j, :],
                                         Tin_b[:, sl], start=False, stop=True)
                        nc.tensor.matmul(PB[:, sl], wts["AB"].rearrange("k (e o) -> k e o", o=68)[:, j, :],
                                         Tin_a[:, sl], start=True, stop=False)
                        nc.tensor.matmul(PB[:, sl], wts["BB"].rearrange("k (e o) -> k e o", o=68)[:, j, :],
                                         Tin_b[:, sl], start=False, stop=True)
                Toa = tpool.tile([128, 512], EW_DT, name=pre + "oa")
                Tob = tpool.tile([69, 512], EW_DT, name=pre + "ob")
                nc.scalar.activation(Toa[:], PA[:], AF.Relu)
                nc.scalar.activation(Tob[0:68, :], PB[:], AF.Relu)
                nc.gpsimd.dma_start(Tob[68:69, :], ones_in[:])
                return Toa, Tob

            T2a, T2b = full_layer(Ta, Tb, "e1")
            T3a, T3b = full_layer(T2a, T2b, "e2")
            if DEBUG:
                nc.gpsimd.dma_start(dbg["dbg_t2a"][:], T2a[:])
                nc.gpsimd.dma_start(dbg["dbg_t3a"][:], T3a[:])

            # L3: in 196, out 98
            P98 = psum_e.tile([98, 512], F32, tag="acc")
            for g in range(128 // G):
                tA = epool.tile([128, G * 98], EW_DT, tag="ew")
                nc.gpsimd.dma_start(tA.rearrange("k (e o) -> k e o", o=98)[:], ew_in["e3A"][:, g * G:(g + 1) * G, :])
                tB = epool.tile([69, G * 98], EW_DT, tag="ew")
                nc.gpsimd.dma_start(tB.rearrange("k (e o) -> k e o", o=98)[:], ew_in["e3B"][:, g * G:(g + 1) * G, :])
                for j in range(G):
                    e = g * G + j
                    sl = slice(4 * e, 4 * e + 4)
                    nc.tensor.matmul(P98[:, sl], tA.rearrange("k (e o) -> k e o", o=98)[:, j, :], T3a[:, sl], start=True, stop=False)
                    nc.tensor.matmul(P98[:, sl], tB.rearrange("k (e o) -> k e o", o=98)[:, j, :], T3b[:, sl], start=False, stop=True)
            T4 = tpool.tile([99, 512], EW_DT)
            nc.scalar.activation(T4[0:98, :], P98[:], AF.Relu)
            nc.gpsimd.dma_start(T4[98:99, :], ones_in[:])

            # L4: in 98(+1), out 24
            P24 = psum_e.tile([24, 512], F32, tag="acc")
            for g in range(128 // G):
                t4 = epool.tile([99, G * 24], EW_DT, tag="ew")
                nc.gpsimd.dma_start(t4.rearrange("k (e o) -> k e o", o=24)[:], ew_in["e4"][:, g * G:(g + 1) * G, :])
                for j in range(G):
                    e = g * G + j
                    sl = slice(4 * e, 4 * e + 4)
                    nc.tensor.matmul(P24[:, sl], t4.rearrange("k (e o) -> k e o", o=24)[:, j, :], T4[:, sl], start=True, stop=True)
            T5 = tpool.tile([25, 512], EW_DT)
            nc.scalar.activation(T5[0:24, :], P24[:], AF.Relu)
            nc.gpsimd.dma_start(T5[24:25, :], ones_in[:])

            # L5: in 24(+1), out 16; final relu (reference relus the stack output)
            P16 = psum_e.tile([16, 512], F32, tag="acc")
            for g in range(128 // G):
                t5 = epool.tile([25, G * 16], EW_DT, tag="ew")
                nc.gpsimd.dma_start(t5.rearrange("k (e o) -> k e o", o=16)[:], ew_in["e5"][:, g * G:(g + 1) * G, :])
                for j in range(G):
                    e = g * G + j
                    sl = slice(4 * e, 4 * e + 4)
                    nc.tensor.matmul(P16[:, sl], t5.rearrange("k (e o) -> k e o", o=16)[:, j, :], T5[:, sl], start=True, stop=True)
            O5 = tpool.tile([16, 512], FW_DT)   # [o, e*4+img]
            nc.scalar.activation(O5[:], P16[:], AF.Relu)

            # ---- Fsb [128e, (img,o)=64]: 4 img-strided transposes of O5
            identb = wpool.tile([128, 128], FW_DT)
            nc.vector.tensor_copy(identb[:], ident[:])
            PT5 = psum_t.tile([128, 64], FW_DT, tag="tr")
            O5v = O5.rearrange("o (e i) -> o e i", i=BL)
            for img in range(BL):
                nc.tensor.transpose(PT5[:, 16 * img:16 * img + 16], O5v[:, :, img], identb[0:16, 0:16])
            Fsb = tpool.tile([128, 64], FW_DT)  # [e, img*16+o]
            nc.vector.tensor_copy(Fsb[:], PT5[:])
            if DEBUG:
                nc.gpsimd.dma_start(dbg["dbg_o5"][:], O5[:])
                nc.gpsimd.dma_start(dbg["dbg_fsb"][:], Fsb[:])

            # =========================================================
            # fusion: stationary = fw chunks (bf16+FWL), moving = [K,4img]
            # layer outputs land as SxT [n%128, mc*4+img] = next moving form
            # =========================================================
            psum_f = psum_c
            ones1 = wpool.tile([1, BL], FW_DT)
            nc.gpsimd.dma_start(ones1[:], ones_in[0:1, 0:BL])

            def nchunks(d):
                return (d + 127) // 128

            # fw1: contraction k=(e,o): 16 o-planes x K=128e; moving = Fsb[:, o::16]
            D1 = FIN_DIMS[1]
            nmc1 = nchunks(D1)   # 16
            S1T = tpool.tile([128, 4 * nmc1], F32)
            P1f = psum_f.tile([128, 4 * nmc1], F32, tag="acc")
            fb1t = fpool.tile([1, D1], FW_DT, tag="fw")
            nc.gpsimd.dma_start(fb1t[:], fb1_in[:])
            Fsbv = Fsb.rearrange("e (i o) -> e i o", o=16)
            for o in range(16):
                w1t = fpool.tile([128, D1], FW_DT, tag="fw")
                nc.gpsimd.dma_start(w1t[:], fw1_in[o, :, :])
                mov = Fsbv[:, :, o]          # [128e, 4img] stride 16
                for mc in range(nmc1):
                    m0, m1 = 128 * mc, min(128 * (mc + 1), D1)
                    nc.tensor.matmul(P1f[0:m1 - m0, 4 * mc:4 * mc + 4], w1t[:, m0:m1], mov,
                                     start=(o == 0 and mc == 0), stop=False)
            for mc in range(nmc1):
                m0, m1 = 128 * mc, min(128 * (mc + 1), D1)
                nc.tensor.matmul(P1f[0:m1 - m0, 4 * mc:4 * mc + 4], fb1t[:, m0:m1], ones1[:],
                                 start=False, stop=(mc == nmc1 - 1))
            nc.scalar.activation(S1T[:], P1f[:], AF.Relu)
            if DEBUG:
                nc.gpsimd.dma_start(dbg["dbg_s1"][:], S1T[:])
            S1b = tpool.tile([128, 4 * nmc1], FW_DT)
            nc.vector.tensor_copy(S1b[:], S1T[:])

            # fw2: straightforward 16kc x 16mc
            D2 = FIN_DIMS[2]
            nmc2 = nchunks(D2)
            S2T = tpool.tile([128, 4 * nmc2], F32)
            P2f = psum_f.tile([128, 4 * nmc2], F32, tag="acc")
            fb2t = fpool.tile([1, D2], FW_DT, tag="fw")
            nc.gpsimd.dma_start(fb2t[:], fb2_in[:])
            for kc in range(nchunks(D1)):
                k0, k1 = 128 * kc, min(128 * (kc + 1), D1)
                w2t = fpool.tile([k1 - k0, D2], FW_DT, tag="fw")
                nc.gpsimd.dma_start(w2t[:], fw2_in[k0:k1, :])
                mov = S1b[0:k1 - k0, 4 * kc:4 * kc + 4]
                for mc in range(nmc2):
                    m0, m1 = 128 * mc, min(128 * (mc + 1), D2)
                    nc.tensor.matmul(P2f[0:m1 - m0, 4 * mc:4 * mc + 4], w2t[:, m0:m1], mov,
                                     start=(kc == 0 and mc == 0), stop=False)
            for mc in range(nmc2):
                m0, m1 = 128 * mc, min(128 * (mc + 1), D2)
                nc.tensor.matmul(P2f[0:m1 - m0, 4 * mc:4 * mc + 4], fb2t[:, m0:m1], ones1[:],
                                 start=False, stop=(mc == nmc2 - 1))
            nc.scalar.activation(S2T[:], P2f[:], AF.Relu)
            if DEBUG:
                nc.gpsimd.dma_start(dbg["dbg_s2"][:], S2T[:])
            S2b = tpool.tile([128, 4 * nmc2], FW_DT)
            nc.vector.tensor_copy(S2b[:], S2T[:])

            # fw3 -> [53, 4]
            D3 = FIN_DIMS[3]
            P3f = psum_f.tile([53, BL], F32, tag="acc")
            fb3t = fpool.tile([1, D3], FW_DT, tag="fw")
            nc.gpsimd.dma_start(fb3t[:], fb3_in[:])
            for kc in range(nchunks(D2)):
                k0, k1 = 128 * kc, min(128 * (kc + 1), D2)
                w3t = fpool.tile([k1 - k0, D3], FW_DT, tag="fw")
                nc.gpsimd.dma_start(w3t[:], fw3_in[k0:k1, :])
                nc.tensor.matmul(P3f[:], w3t[:], S2b[0:k1 - k0, 4 * kc:4 * kc + 4],
                                 start=(kc == 0), stop=False)
            nc.tensor.matmul(P3f[:], fb3t[:], ones1[:], start=False, stop=True)
            S3 = tpool.tile([53, BL], F32)
            nc.scalar.activation(S3[:], P3f[:], AF.Copy)
            nc.gpsimd.dma_start(y_out[:].rearrange("b o -> o b"), S3[:])

    # tilefix patch
    orig = nc.to_json_bytes
    nc.to_json_bytes = lambda: _fix_bir_json(orig())
    return nc


def ew_shapes_cw(i):
    return [128, 9, 128]


# ---------------------------------------------------------------- host prep
def _host_prep(inputs):
    f = lambda a: np.asarray(a, dtype=np.float32)
    cws = [f(inputs[f"cw{i+1}"]) for i in range(4)]
    cbs = [f(inputs[f"cb{i+1}"]) for i in range(4)]
    base = {}
    base["ident"] = np.eye(128, dtype=np.float32)

    # conv weights: block-diagonal image-pair lhsT layouts [128, 9, M]
    # cw1: pair pr at rows 64pr..64pr+5 (3ch x 2img), cols 32s+o
    t = np.zeros((128, 9, 128), np.float32)
    lhs1 = cws[0].transpose(1, 2, 3, 0).reshape(3, 9, 32)   # [cin, k, cout]
    for img in range(4):
        t[3 * img:3 * img + 3, :, 32 * img:32 * img + 32] = lhs1
    base["cwr0"] = t
    # cw2: K=64 (2img x 32ch) block-diag: rows 32s+c -> cols 64s+o; replicated per pair
    t = np.zeros((128, 9, 128), np.float32)
    lhs2 = cws[1].transpose(1, 2, 3, 0).reshape(32, 9, 64)
    for s in range(2):
        t[32 * s:32 * s + 32, :, 64 * s:64 * s + 64] = lhs2
    t[64:128] = t[0:64]
    base["cwr1"] = t
    # cw3: K=64 per img at rows 64sl
    t = np.zeros((128, 9, 128), np.float32)
    lhs3 = cws[2].transpose(1, 2, 3, 0).reshape(64, 9, 128)
    t[0:64] = lhs3
    t[64:128] = lhs3
    base["cwr2"] = t
    # cw4: K=128
    base["cwr3"] = np.ascontiguousarray(cws[3].transpose(1, 2, 3, 0).reshape(128, 9, 128))
    # conv biases
    cbr = []
    for i, cbi in enumerate(cbs):
        t = np.zeros((128, 1), np.float32)
        if i == 0:
            for img in range(BL):
                t[32 * img:32 * img + 32, 0] = cbi
        elif i == 1:
            for s in range(2):
                t[64 * s:64 * s + 64, 0] = cbi
        else:
            t[:, 0] = cbi
        base[f"cbr{i}"] = t

    base["awr"] = f(inputs["aw"])[:, :, 0]
    base["abr"] = f(inputs["ab"]).reshape(128, 1)

    # expert weights, K-major with bias rows
    ew = [f(inputs[f"ew{i+1}"]) for i in range(5)]
    eb = [f(inputs[f"eb{i+1}"]) for i in range(5)]
    km = lambda a: np.ascontiguousarray(a.transpose(1, 0, 2))  # [K, e, o]

    def aug(wB, bias):  # append bias row: [Kb, e, o] + [e, o] -> [Kb+1, e, o]
        return np.concatenate([wB, bias[None, :, :]], axis=0)

    for li, pre in ((0, "e1"), (1, "e2")):
        w = ew[li]
        base[pre + "AA"] = km(w[:, 0:128, 0:128]).astype(EW_NP)
        base[pre + "AB"] = km(w[:, 0:128, 128:196]).astype(EW_NP)
        base[pre + "BA"] = aug(km(w[:, 128:196, 0:128]), eb[li][:, 0:128]).astype(EW_NP)
        base[pre + "BB"] = aug(km(w[:, 128:196, 128:196]), eb[li][:, 128:196]).astype(EW_NP)
    base["e3A"] = km(ew[2][:, 0:128, :]).astype(EW_NP)
    base["e3B"] = aug(km(ew[2][:, 128:196, :]), eb[2]).astype(EW_NP)
    base["e4"] = aug(km(ew[3]), eb[3]).astype(EW_NP)
    base["e5"] = aug(km(ew[4]), eb[4]).astype(EW_NP)

    fw1 = f(inputs["fw1"])
    # fw1p[o, e, n] = fw1[e*16+o, n]
    fw1p = np.ascontiguousarray(fw1.reshape(128, 16, FIN_DIMS[1]).transpose(1, 0, 2))
    base["fw1p"] = fw1p.astype(FW_NP)
    base["fb1r"] = f(inputs["fb1"]).reshape(1, -1).astype(FW_NP)
    base["fw2r"] = f(inputs["fw2"]).astype(FW_NP)
    base["fb2r"] = f(inputs["fb2"]).reshape(1, -1).astype(FW_NP)
    base["fw3r"] = f(inputs["fw3"]).astype(FW_NP)
    base["fb3r"] = f(inputs["fb3"]).reshape(1, -1).astype(FW_NP)
    base["onesrow"] = np.ones((1, 512), EW_NP)
    return base


def kernel(**inputs):
    global _BUILT
    if _BUILT is None:
        _BUILT = _build()
    nc = _BUILT
    base = _host_prep(inputs)
    x = np.asarray(inputs["x"], dtype=np.float32)
    in_maps = []
    for c in range(NCORES):
        m = dict(base)
        m["x"] = np.ascontiguousarray(x[c * BL:(c + 1) * BL])
        in_maps.append(m)
    res = run_bass_kernel_spmd(nc, in_maps, list(range(NCORES)))
    return np.concatenate([res.results[c]["y"] for c in range(NCORES)], axis=0)


if __name__ == "__main__":
    rng = np.random.default_rng(0)
    fake = {}
    # quick shape smoke with random inputs
    fake["x"] = rng.standard_normal((B, 3, H, W), dtype=np.float32)
    for i in range(4):
        cin, cout = CHANS[i], CHANS[i + 1]
        fake[f"cw{i+1}"] = rng.standard_normal((cout, cin, 3, 3), dtype=np.float32)
        fake[f"cb{i+1}"] = np.zeros(cout, np.float32)
    fake["aw"] = rng.standard_normal((NF, FLAT, 1), dtype=np.float32)
    fake["ab"] = np.zeros((NF, 1), np.float32)
    for i in range(5):
        di, do = EXP_DIMS[i], EXP_DIMS[i + 1]
        fake[f"ew{i+1}"] = rng.standard_normal((NF, di, do), dtype=np.float32)
        fake[f"eb{i+1}"] = np.zeros((NF, do), np.float32)
    for i in range(3):
        di, do = FIN_DIMS[i], FIN_DIMS[i + 1]
        fake[f"fw{i+1}"] = rng.standard_normal((di, do), dtype=np.float32)
        fake[f"fb{i+1}"] = np.zeros(do, np.float32)
    y = kernel(**fake)
    print("y", y.shape, y.dtype)



# revision 19
# speedup vs baseline: 1.9435x; 1.9435x over previous
"""TRN2 Bass kernel for nn_CardClassifier.

CNN(4x conv3x3+relu+maxpool2) -> per-feature sigmoid attention ->
128 stacked expert MLPs -> fusion MLP (2048->2038->2028->53).

Distribution: data-parallel convs (8 cores x 4 images), then AllToAll to
expert parallelism (16 experts/core x 32 images), K-sharded fusion with
two ReduceScatters; final 53-dim partials summed on the host.

Conv engines: tap-folded K packing (im2col replicas built by strided DMA
from a host-padded input / shifted SBUF-SBUF copies), bf16 weights and
activations, fp32 PSUM. Bias+relu folded after each maxpool (commute).
"""

import sys

sys.path.insert(0, "/opt/trn_rl_repo")

import json as _json
import contextlib
import numpy as np
import ml_dtypes

import bass_rust
import concourse.bass as bass
import concourse.mybir as mybir
from concourse import tile
from concourse.bass_utils import run_bass_kernel_spmd

F32 = mybir.dt.float32
BF16 = mybir.dt.bfloat16
AF = mybir.ActivationFunctionType
ALU = None  # filled lazily
BF = ml_dtypes.bfloat16

B, H, W = 32, 224, 224
NCORES, BL = 8, 4
NF, FLAT = 128, 196
EXP_DIMS = [196, 196, 196, 98, 24, 16]
FIN = [2048, 2038, 2028, 53]
EPC = 16  # experts per core

_BUILT = None
RG = [list(range(NCORES))]


# ---------------------------------------------------------------- tilefix
def _fix_bir_json(raw: bytes) -> bytes:
    """This walrus build allows at most 1 sync-wait per instruction; Tile's
    tail drain can carry more. Split extras onto NoOp carriers."""
    d = _json.loads(raw)
    k = 0
    for fn in d.get("functions", []):
        for blk in fn.get("blocks", []):
            out = []
            for inst in blk["instructions"]:
                si = inst.get("sync_info")
                waits = (si or {}).get("on_wait") or []
                if len(waits) > 1:
                    for wchunk in waits[:-1]:
                        out.append({
                            "debug": inst.get("debug", 0),
                            "engine": inst["engine"],
                            "ins": [], "outs": [],
                            "name": f"NOPW-{k}",
                            "opcode": "NoOp",
                            "sync_info": {"on_update": [], "on_wait": [wchunk]},
                        })
                        k += 1
                    si["on_wait"] = waits[-1:]
                out.append(inst)
            blk["instructions"] = out
    return _json.dumps(d).encode()


def _vp(dims):
    return bass_rust.VecI64Pair(dims)


# ---------------------------------------------------------------- build
def _build():
    global ALU
    from concourse.alu_op_type import AluOpType as ALU_

    ALU = ALU_
    nc = bass.Bass("TRN2", target_bir_lowering=False, debug=False,
                   num_devices=NCORES)

    dp = lambda name, shape, dt: nc.declare_dram_parameter(name, list(shape), dt, isOutput=False)

    xp = dp("xpad", [BL * 3 * 226 * 226 + 2], BF16)  # +2: kx-merged rows over-read tail
    cw1p = dp("cw1p", [108, 128], BF16)
    cb1r = dp("cb1r", [128, 1], F32)
    cw2p = dp("cw2p", [96, 192], BF16)
    cb2r = dp("cb2r", [64, 1], F32)
    cw3ap = dp("cw3ap", [128, 384], BF16)
    cw3bp = dp("cw3bp", [64, 384], BF16)
    cb3r = dp("cb3r", [128, 1], F32)
    cw4p = dp("cw4p", [128, 1152], BF16)
    cb4r = dp("cb4r", [128, 1], F32)
    awr = dp("awr", [128, FLAT], BF16)
    abr = dp("abr", [128, 1], F32)

    ew_shapes = {
        "e1AA": (128, EPC * 128), "e1AB": (128, EPC * 68),
        "e1BA": (69, EPC * 128), "e1BB": (69, EPC * 68),
        "e2AA": (128, EPC * 128), "e2AB": (128, EPC * 68),
        "e2BA": (69, EPC * 128), "e2BB": (69, EPC * 68),
        "e3A": (128, EPC * 98), "e3B": (69, EPC * 98),
        "e4": (99, EPC * 24),
        "e5p": (64, 256),
    }
    ew_in = {k: dp(k, list(s), BF16) for k, s in ew_shapes.items()}

    fw1_in = dp("fw1p", [2, 128, 2048], BF16)
    fb1_in = dp("fb1d8", [1, 2048], BF16)
    fw2_in = dp("fw2p", [2, 128, 2048], BF16)
    fb2_in = dp("fb2d8", [1, 2048], BF16)
    fw3_in = dp("fw3p", [2, 128, 53], BF16)
    fb3_in = dp("fb3d8", [1, 53], BF16)
    ones_in = dp("onesrow", [1, 512], BF16)
    t5f_in = dp("t5fill", [64, 256], BF16)

    y_out = nc.declare_dram_parameter("y", [32, 53], F32, isOutput=True)

    with tile.TileContext(nc, pool_alloc_mode="queue") as tc:
        stk = contextlib.ExitStack()
        with stk:
            # ---- persistent consts
            wpool = stk.enter_context(tc.tile_pool(name="wconst", bufs=1))
            cw1 = wpool.tile([108, 128], BF16)
            nc.sync.dma_start(cw1[:], cw1p[:])
            cw2 = wpool.tile([96, 192], BF16)
            nc.sync.dma_start(cw2[:], cw2p[:])
            cw3a = wpool.tile([128, 384], BF16)
            nc.sync.dma_start(cw3a[:], cw3ap[:])
            cw3b = wpool.tile([64, 384], BF16)
            nc.sync.dma_start(cw3b[:], cw3bp[:])
            cw4 = wpool.tile([128, 1152], BF16)
            nc.sync.dma_start(cw4[:], cw4p[:])
            cb1 = wpool.tile([128, 1], F32)
            nc.sync.dma_start(cb1[:], cb1r[:])
            cb2 = wpool.tile([64, 1], F32)
            nc.sync.dma_start(cb2[:], cb2r[:])
            cb3 = wpool.tile([128, 1], F32)
            nc.sync.dma_start(cb3[:], cb3r[:])
            cb4 = wpool.tile([128, 1], F32)
            nc.sync.dma_start(cb4[:], cb4r[:])
            awsb = wpool.tile([128, FLAT], BF16)
            nc.sync.dma_start(awsb[:], awr[:])
            absb = wpool.tile([128, 1], F32)
            nc.sync.dma_start(absb[:], abr[:])
            ones1 = wpool.tile([1, 32], BF16)
            nc.vector.memset(ones1[:], 1.0)

            hpool = stk.enter_context(tc.tile_pool(name="hp", bufs=1))
            Ht = hpool.tile([128, BL * FLAT], BF16)
            HB = hpool.tile([128, BL * FLAT], BF16)

            # conv2 replica tiles (pool opened before a1r for LIFO release)
            a2pool = stk.enter_context(tc.tile_pool(name="a2r", bufs=1))
            A2Ra = [a2pool.tile([128, 56 * 58], BF16, name=f"a2a{i}")
                    for i in range(BL)]
            A2Rb = [a2pool.tile([64, 56 * 58], BF16, name=f"a2b{i}")
                    for i in range(BL)]

            # =========================================================
            # conv1: 3->32, im2col K=108 (9 taps x 4img x 3ch), M=128
            # 14 strips of 16 output rows; pool+bias+relu -> pm
            # =========================================================
            a1stk = contextlib.ExitStack()
            a1pool = a1stk.enter_context(tc.tile_pool(name="a1r", bufs=1))
            A1R = [a1pool.tile([96, 112 * 114], BF16, name=f"a1r{i}")
                   for i in range(BL)]

            with tc.tile_pool(name="c1x", bufs=2) as xpool, \
                 tc.tile_pool(name="c1v", bufs=2) as vpool, \
                 tc.tile_pool(name="c1m", bufs=2) as mpool, \
                 tc.tile_pool(name="c1pm", bufs=1) as pmpool, \
                 tc.tile_pool(name="ps1", bufs=6, space="PSUM") as psc:
                pm = pmpool.tile([128, 112 * 112], BF16)
                pmv = pm.rearrange("p (r c) -> p r c", c=112)
                for s in range(14):
                    r0 = 16 * s
                    X9 = xpool.tile([108, 16 * 226], BF16, tag="x9")
                    for ky in range(3):
                        src = xp[:]
                        src.ap = _vp([[1, 3], [51076, 12], [1, 3616]])
                        src.offset = src.offset + (r0 + ky) * 226
                        nc.sync.dma_start(X9[36 * ky:36 * ky + 36, :], src)
                    X9v = X9.rearrange("p (r c) -> p r c", c=226)
                    pvs = vpool.tile([128, 8 * 112], F32, tag="pvs")
                    pvsv = pvs.rearrange("p (r c) -> p r c", c=112)
                    for t in range(8):
                        P = psc.tile([128, 448], F32, tag="acc")
                        nc.tensor.matmul(P[:], cw1[:],
                                         X9v[:, 2 * t:2 * t + 2, 0:224],
                                         start=True, stop=True)
                        nc.vector.tensor_reduce(
                            pvsv[:, t:t + 1, :],
                            P.rearrange("p (r c t) -> p c r t", r=2, c=112, t=2),
                            axis=mybir.AxisListType.XY, op=ALU.max)
                    nc.vector.tensor_scalar(pmv[:, 8 * s:8 * s + 8, :], pvsv[:],
                                            cb1[:, 0:1], 0.0,
                                            op0=ALU.add, op1=ALU.max)
                # replicate pm into per-image (ky,ch) layouts
                for i in range(BL):
                    av = A1R[i].rearrange("p (r c) -> p r c", c=114)
                    nc.vector.memset(av[:, :, 0], 0.0)
                    nc.vector.memset(av[:, :, 113], 0.0)
                    nc.vector.memset(av[0:32, 0, :], 0.0)
                    nc.vector.memset(av[64:96, 111, :], 0.0)
                    sp = pmv[32 * i:32 * i + 32, :, :]
                    nc.sync.dma_start(av[32:64, 0:112, 1:113], sp)
                    nc.sync.dma_start(av[0:32, 1:112, 1:113], sp[:, 0:111, :])
                    nc.sync.dma_start(av[64:96, 0:111, 1:113], sp[:, 1:112, :])

            # =========================================================
            # conv2: 32->64 per img, K=96 (3ky x 32ch), 3 kx passes, M=64
            # =========================================================
            with tc.tile_pool(name="c2v", bufs=1) as v2pool, \
                 tc.tile_pool(name="c2m", bufs=1) as m2pool, \
                 tc.tile_pool(name="ps2", bufs=6, space="PSUM") as psc2:
                for i in range(BL):
                    av = A1R[i].rearrange("p (r c) -> p r c", c=114)
                    pvs2 = v2pool.tile([64, 56 * 56], F32, tag="pvs2")
                    p2v = pvs2.rearrange("p (r c) -> p r c", c=56)
                    for t in range(28):
                        P = psc2.tile([64, 448], F32, tag="acc2")
                        for kx in range(3):
                            nc.tensor.matmul(P[:], cw2[:, 64 * kx:64 * kx + 64],
                                             av[:, 4 * t:4 * t + 4, kx:kx + 112],
                                             start=(kx == 0), stop=(kx == 2))
                        nc.vector.tensor_reduce(
                            p2v[:, 2 * t:2 * t + 2, :],
                            P.rearrange("p (r a c b) -> p r c a b",
                                        r=2, a=2, c=56, b=2),
                            axis=mybir.AxisListType.XY, op=ALU.max)
                    pm2 = m2pool.tile([64, 56 * 56], BF16, tag="pm2")
                    nc.vector.tensor_scalar(pm2[:], pvs2[:], cb2[:, 0:1], 0.0,
                                            op0=ALU.add, op1=ALU.max)
                    pm2v = pm2.rearrange("p (r c) -> p r c", c=56)
                    aav = A2Ra[i].rearrange("p (r c) -> p r c", c=58)
                    abv = A2Rb[i].rearrange("p (r c) -> p r c", c=58)
                    nc.vector.memset(aav[:, :, 0], 0.0)
                    nc.vector.memset(aav[:, :, 57], 0.0)
                    nc.vector.memset(abv[:, :, 0], 0.0)
                    nc.vector.memset(abv[:, :, 57], 0.0)
                    nc.vector.memset(aav[0:64, 0, :], 0.0)
                    nc.vector.memset(abv[:, 55, :], 0.0)
                    nc.sync.dma_start(aav[64:128, 0:56, 1:57], pm2v[:])
                    nc.sync.dma_start(aav[0:64, 1:56, 1:57], pm2v[:, 0:55, :])
                    nc.sync.dma_start(abv[:, 0:55, 1:57], pm2v[:, 1:56, :])
            a1stk.close()

            # ---- expert + fusion weights to SBUF (during conv3/4;
            # pool opened after A1R frees so the ring has room)
            ewfpool = stk.enter_context(tc.tile_pool(name="ewf", bufs=1))
            ewsb = {}
            for k, s in ew_shapes.items():
                t = ewfpool.tile(list(s), BF16, name=k + "sb")
                nc.sync.dma_start(t[:], ew_in[k][:])
                ewsb[k] = t
            fw1sb = ewfpool.tile([128, 4096], BF16)
            s1 = fw1_in[:]
            s1.ap = _vp([[2048, 128], [262144, 2], [1, 2048]])
            nc.sync.dma_start(fw1sb[:], s1)
            fw2sb = ewfpool.tile([128, 4096], BF16)
            s2 = fw2_in[:]
            s2.ap = _vp([[2048, 128], [262144, 2], [1, 2048]])
            nc.sync.dma_start(fw2sb[:], s2)
            fw3sb = ewfpool.tile([128, 106], BF16)
            s3 = fw3_in[:]
            s3.ap = _vp([[53, 128], [6784, 2], [1, 53]])
            nc.sync.dma_start(fw3sb[:], s3)
            fb1sb = ewfpool.tile([1, 2048], BF16)
            nc.sync.dma_start(fb1sb[:], fb1_in[:])
            fb2sb = ewfpool.tile([1, 2048], BF16)
            nc.sync.dma_start(fb2sb[:], fb2_in[:])
            fb3sb = ewfpool.tile([1, 53], BF16)
            nc.sync.dma_start(fb3sb[:], fb3_in[:])

            # =========================================================
            # conv3: 64->128 per img, K=128 (2ky x 64ch) + K=64 (ky2)
            # =========================================================
            a3pool = stk.enter_context(tc.tile_pool(name="a3m", bufs=1))
            A3 = [a3pool.tile([128, 30 * 30], BF16, name=f"a3_{i}")
                  for i in range(BL)]

            with tc.tile_pool(name="c3v", bufs=2) as v3pool, \
                 tc.tile_pool(name="ps3", bufs=6, space="PSUM") as psc3:
                for i in range(BL):
                    aav = A2Ra[i].rearrange("p (r c) -> p r c", c=58)
                    abv = A2Rb[i].rearrange("p (r c) -> p r c", c=58)
                    pvs3 = v3pool.tile([128, 28 * 28], F32, tag="pvs3")
                    p3v = pvs3.rearrange("p (r c) -> p r c", c=28)
                    for t in range(7):
                        P = psc3.tile([128, 448], F32, tag="acc3")
                        for kx in range(3):
                            nc.tensor.matmul(P[:], cw3a[:, 128 * kx:128 * kx + 128],
                                             aav[:, 8 * t:8 * t + 8, kx:kx + 56],
                                             start=(kx == 0), stop=False)
                            nc.tensor.matmul(P[:], cw3b[:, 128 * kx:128 * kx + 128],
                                             abv[:, 8 * t:8 * t + 8, kx:kx + 56],
                                             start=False, stop=(kx == 2))
                        nc.vector.tensor_reduce(
                            p3v[:, 4 * t:4 * t + 4, :],
                            P.rearrange("p (r a c b) -> p r c a b",
                                        r=4, a=2, c=28, b=2),
                            axis=mybir.AxisListType.XY, op=ALU.max)
                    a3v = A3[i].rearrange("p (r c) -> p r c", c=30)
                    nc.vector.memset(a3v[:, 0, :], 0.0)
                    nc.vector.memset(a3v[:, 29, :], 0.0)
                    nc.vector.memset(a3v[:, :, 0], 0.0)
                    nc.vector.memset(a3v[:, :, 29], 0.0)
                    nc.vector.tensor_scalar(a3v[:, 1:29, 1:29], p3v[:],
                                            cb3[:, 0:1], 0.0,
                                            op0=ALU.add, op1=ALU.max)

                # =====================================================
                # conv4: 128->128 per img, K=128, 9 taps via views
                # =====================================================
                for i in range(BL):
                    a3v = A3[i].rearrange("p (r c) -> p r c", c=30)
                    pvs4 = v3pool.tile([128, 14 * 14], F32, tag="pvs4")
                    p4v = pvs4.rearrange("p (r c) -> p r c", c=14)
                    for t, (rb, nr) in enumerate(((0, 16), (16, 12))):
                        P = psc3.tile([128, nr * 28], F32, tag="acc3")
                        for k in range(9):
                            ky, kx = divmod(k, 3)
                            nc.tensor.matmul(P[:], cw4[:, 128 * k:128 * k + 128],
                                             a3v[:, rb + ky:rb + ky + nr, kx:kx + 28],
                                             start=(k == 0), stop=(k == 8))
                        nc.vector.tensor_reduce(
                            p4v[:, rb // 2:rb // 2 + nr // 2, :],
                            P.rearrange("p (r a c b) -> p r c a b",
                                        r=nr // 2, a=2, c=14, b=2),
                            axis=mybir.AxisListType.XY, op=ALU.max)
                    nc.vector.tensor_scalar(Ht[:, FLAT * i:FLAT * (i + 1)], pvs4[:],
                                            cb4[:, 0:1], 0.0,
                                            op0=ALU.add, op1=ALU.max)

            # =========================================================
            # attention: att = sigmoid(feats . aw + ab); h = feats * att
            # =========================================================
            with tc.tile_pool(name="att", bufs=2) as atp:
                for i in range(BL):
                    sl = slice(FLAT * i, FLAT * (i + 1))
                    tmp = atp.tile([128, FLAT], F32, tag="tmp")
                    nc.vector.tensor_tensor(tmp[:], Ht[:, sl], awsb[:], op=ALU.mult)
                    attv = atp.tile([128, 1], F32, tag="av")
                    nc.vector.tensor_reduce(attv[:], tmp[:],
                                            axis=mybir.AxisListType.X, op=ALU.add)
                    atts = atp.tile([128, 1], F32, tag="as")
                    nc.scalar.activation(atts[:], attv[:], AF.Sigmoid, bias=absb[:])
                    nc.vector.tensor_scalar(HB[:, sl], Ht[:, sl], atts[:, 0:1],
                                            None, op0=ALU.mult)

            # =========================================================
            # AllToAll: [128e, 4i x 256fpad] -> [8s x 16e, 4i x 256fpad]
            # =========================================================
            dram = stk.enter_context(tc.tile_pool(name="dram", bufs=1, space="DRAM"))
            in_b = dram.tile([128, 1024], BF16)
            out_b = dram.tile([128, 1024], BF16)
            ibv = in_b.rearrange("p (i f) -> p i f", f=256)
            nc.sync.dma_start(ibv[:, :, 0:FLAT],
                              HB.rearrange("p (i f) -> p i f", f=FLAT)[:])
            nc.gpsimd.collective_compute(
                "AllToAll", mybir.AluOpType.bypass, replica_groups=RG,
                ins=[in_b.opt()], outs=[out_b.opt()])

            # transpose to [f, (s,e,i)] then relabel cols to (e, g=4s+i)
            tpool = stk.enter_context(tc.tile_pool(name="texp", bufs=1))
            TAraw = tpool.tile([128, 512], BF16)
            TBraw = tpool.tile([128, 512], BF16)
            for blk, dst in ((0, TAraw), (1, TBraw)):
                src = out_b[:]
                src.ap = _vp([[256, 512], [1, 128]])
                src.offset = src.offset + 128 * blk
                nc.sync.dma_start_transpose(dst[:], src)
            TA1 = tpool.tile([128, 512], BF16)
            TB1 = tpool.tile([69, 512], BF16)
            TAv = TA1.rearrange("p (e g) -> p e g", g=32)
            TBv = TB1.rearrange("p (e g) -> p e g", g=32)
            for s in range(8):
                nc.vector.tensor_copy(
                    TAv[:, :, 4 * s:4 * s + 4],
                    TAraw[:, 64 * s:64 * s + 64].rearrange("p (e i) -> p e i", i=4))
                nc.vector.tensor_copy(
                    TBv[0:68, :, 4 * s:4 * s + 4],
                    TBraw[0:68, 64 * s:64 * s + 64].rearrange("p (e i) -> p e i", i=4))
            nc.sync.dma_start(TB1[68:69, :], ones_in[:])

            # =========================================================
            # experts: 16 local experts x 32 imgs, weight-stationary
            # =========================================================
            pse = stk.enter_context(tc.tile_pool(name="pse", bufs=5, space="PSUM"))
            pse2 = stk.enter_context(tc.tile_pool(name="pse2", bufs=2, space="PSUM"))

            def elayer(TAi, TBi, pre):
                PA = pse.tile([128, 512], F32, tag="pacc")
                PB = pse.tile([68, 512], F32, tag="pacc")
                wAA, wAB = ewsb[pre + "AA"], ewsb[pre + "AB"]
                wBA, wBB = ewsb[pre + "BA"], ewsb[pre + "BB"]
                for e in range(EPC):
                    sl = slice(32 * e, 32 * e + 32)
                    nc.tensor.matmul(PA[:, sl], wAA[:, 128 * e:128 * e + 128],
                                     TAi[:, sl], start=True, stop=False)
                    nc.tensor.matmul(PA[:, sl], wBA[:, 128 * e:128 * e + 128],
                                     TBi[:, sl], start=False, stop=True)
                    nc.tensor.matmul(PB[:, sl], wAB[:, 68 * e:68 * e + 68],
                                     TAi[:, sl], start=True, stop=False)
                    nc.tensor.matmul(PB[:, sl], wBB[:, 68 * e:68 * e + 68],
                                     TBi[:, sl], start=False, stop=True)
                TAo = tpool.tile([128, 512], BF16, name=pre + "oa")
                TBo = tpool.tile([69, 512], BF16, name=pre + "ob")
                nc.scalar.activation(TAo[:], PA[:], AF.Relu)
                nc.scalar.activation(TBo[0:68, :], PB[:], AF.Relu)
                nc.sync.dma_start(TBo[68:69, :], ones_in[:])
                return TAo, TBo

            TA2, TB2 = elayer(TA1, TB1, "e1")
            TA3, TB3 = elayer(TA2, TB2, "e2")

            P98 = pse.tile([98, 512], F32, tag="pacc")
            for e in range(EPC):
                sl = slice(32 * e, 32 * e + 32)
                nc.tensor.matmul(P98[:, sl], ewsb["e3A"][:, 98 * e:98 * e + 98],
                                 TA3[:, sl], start=True, stop=False)
                nc.tensor.matmul(P98[:, sl], ewsb["e3B"][:, 98 * e:98 * e + 98],
                                 TB3[:, sl], start=False, stop=True)
            T4 = tpool.tile([99, 512], BF16)
            nc.scalar.activation(T4[0:98, :], P98[:], AF.Relu)
            nc.sync.dma_start(T4[98:99, :], ones_in[:])

            P24 = pse.tile([24, 512], F32, tag="pacc")
            for e in range(EPC):
                sl = slice(32 * e, 32 * e + 32)
                nc.tensor.matmul(P24[:, sl], ewsb["e4"][:, 24 * e:24 * e + 24],
                                 T4[:, sl], start=True, stop=True)

            # pair layout for L5: rows 0..24 even expert, 32..56 odd
            # (zeros + ones-rows prefilled from DRAM, data rows overwritten)
            T5R = tpool.tile([64, 256], BF16)
            nc.sync.dma_start(T5R[:], t5f_in[:])
            P24v = P24.rearrange("p (e g) -> p e g", g=32)
            T5Rv = T5R.rearrange("p (q g) -> p q g", g=32)
            nc.scalar.activation(T5Rv[0:24, :, :], P24v[0:24, 0:16:2, :], AF.Relu)
            nc.scalar.activation(T5Rv[32:56, :, :], P24v[0:24, 1:16:2, :], AF.Relu)

            S5 = pse2.tile([128, 64], F32, tag="ps5")
            for p in range(8):
                nc.tensor.matmul(S5[32 * (p % 4):32 * (p % 4) + 32,
                                    32 * (p // 4):32 * (p // 4) + 32],
                                 ewsb["e5p"][:, 32 * p:32 * p + 32],
                                 T5R[:, 32 * p:32 * p + 32],
                                 start=True, stop=True,
                                 tile_position=(0, 32 * (p % 4)))
            SF = tpool.tile([128, 64], BF16)
            nc.scalar.activation(SF[:], S5[:], AF.Relu)

            # =========================================================
            # fusion: K-sharded partials + ReduceScatter x2, host sum
            # =========================================================
            rs1i = dram.tile([2048, 32], BF16)
            rs1o = dram.tile([256, 32], BF16)
            rs2i = dram.tile([2048, 32], BF16)
            rs2o = dram.tile([256, 32], BF16)

            P1 = pse.tile([128, 512], F32, tag="pacc")
            for mc in range(16):
                msl = slice(32 * mc, 32 * mc + 32)
                for g in range(2):
                    nc.tensor.matmul(P1[:, msl],
                                     fw1sb[:, 2048 * g + 128 * mc:2048 * g + 128 * mc + 128],
                                     SF[:, 32 * g:32 * g + 32],
                                     start=(g == 0), stop=False)
                nc.tensor.matmul(P1[:, msl], fb1sb[:, 128 * mc:128 * mc + 128],
                                 ones1[:], start=False, stop=True)
            S1pre = tpool.tile([128, 512], BF16)
            nc.scalar.activation(S1pre[:], P1[:], AF.Copy)
            d1 = rs1i[:]
            d1.ap = _vp([[32, 128], [4096, 16], [1, 32]])
            nc.sync.dma_start(d1, S1pre[:])
            nc.gpsimd.collective_compute(
                "ReduceScatter", mybir.AluOpType.add, replica_groups=RG,
                ins=[rs1i.opt()], outs=[rs1o.opt()])
            S1c = tpool.tile([128, 64], BF16)
            sr = rs1o[:]
            sr.ap = _vp([[32, 128], [4096, 2], [1, 32]])
            nc.sync.dma_start(S1c[:], sr)
            S1 = tpool.tile([128, 64], BF16)
            nc.scalar.activation(S1[:], S1c[:], AF.Relu)

            P2 = pse.tile([128, 512], F32, tag="pacc")
            for mc in range(16):
                msl = slice(32 * mc, 32 * mc + 32)
                for kc in range(2):
                    nc.tensor.matmul(P2[:, msl],
                                     fw2sb[:, 2048 * kc + 128 * mc:2048 * kc + 128 * mc + 128],
                                     S1[:, 32 * kc:32 * kc + 32],
                                     start=(kc == 0), stop=False)
                nc.tensor.matmul(P2[:, msl], fb2sb[:, 128 * mc:128 * mc + 128],
                                 ones1[:], start=False, stop=True)
            S2pre = tpool.tile([128, 512], BF16)
            nc.scalar.activation(S2pre[:], P2[:], AF.Copy)
            d2 = rs2i[:]
            d2.ap = _vp([[32, 128], [4096, 16], [1, 32]])
            nc.sync.dma_start(d2, S2pre[:])
            nc.gpsimd.collective_compute(
                "ReduceScatter", mybir.AluOpType.add, replica_groups=RG,
                ins=[rs2i.opt()], outs=[rs2o.opt()])
            S2c = tpool.tile([128, 64], BF16)
            sr2 = rs2o[:]
            sr2.ap = _vp([[32, 128], [4096, 2], [1, 32]])
            nc.sync.dma_start(S2c[:], sr2)
            S2 = tpool.tile([128, 64], BF16)
            nc.scalar.activation(S2[:], S2c[:], AF.Relu)

            P3 = pse2.tile([53, 32], F32, tag="ps5")
            for kc in range(2):
                nc.tensor.matmul(P3[:], fw3sb[:, 53 * kc:53 * kc + 53],
                                 S2[:, 32 * kc:32 * kc + 32],
                                 start=(kc == 0), stop=False)
            nc.tensor.matmul(P3[:], fb3sb[:], ones1[:], start=False, stop=True)
            S3 = tpool.tile([53, 32], F32)
            nc.scalar.activation(S3[:], P3[:], AF.Copy)
            nc.sync.dma_start(y_out[:].rearrange("b o -> o b"), S3[:])

    orig = nc.to_json_bytes
    nc.to_json_bytes = lambda: _fix_bir_json(orig())
    return nc


# ---------------------------------------------------------------- host prep
def _host_shared(inputs):
    f32 = np.float32
    cw = [np.asarray(inputs[f"cw{i+1}"], f32) for i in range(4)]
    cb = [np.asarray(inputs[f"cb{i+1}"], f32) for i in range(4)]
    d = {}
    t = np.zeros((108, 128), f32)
    for ky in range(3):
        for kx in range(3):
            blk = cw[0][:, :, ky, kx].T
            for img in range(4):
                r = (ky * 3 + kx) * 12 + img * 3
                t[r:r + 3, img * 32:(img + 1) * 32] = blk
    d["cw1p"] = t.astype(BF)
    d["cb1r"] = np.tile(cb[0], 4).reshape(128, 1)
    t = np.zeros((96, 192), f32)
    for ky in range(3):
        for kx in range(3):
            t[ky * 32:(ky + 1) * 32, kx * 64:(kx + 1) * 64] = cw[1][:, :, ky, kx].T
    d["cw2p"] = t.astype(BF)
    d["cb2r"] = cb[1].reshape(64, 1)
    ta = np.zeros((128, 384), f32)
    tb = np.zeros((64, 384), f32)
    for kx in range(3):
        for ky in range(2):
            ta[ky * 64:(ky + 1) * 64, kx * 128:(kx + 1) * 128] = cw[2][:, :, ky, kx].T
        tb[:, kx * 128:(kx + 1) * 128] = cw[2][:, :, 2, kx].T
    d["cw3ap"] = ta.astype(BF)
    d["cw3bp"] = tb.astype(BF)
    d["cb3r"] = cb[2].reshape(128, 1)
    t = np.zeros((128, 1152), f32)
    for k in range(9):
        ky, kx = divmod(k, 3)
        t[:, k * 128:(k + 1) * 128] = cw[3][:, :, ky, kx].T
    d["cw4p"] = t.astype(BF)
    d["cb4r"] = cb[3].reshape(128, 1)
    d["awr"] = np.asarray(inputs["aw"], f32)[:, :, 0].astype(BF)
    d["abr"] = np.asarray(inputs["ab"], f32).reshape(128, 1)
    return d


def _host_shard(inputs, c):
    f32 = np.float32
    E0 = EPC * c
    ew = [np.asarray(inputs[f"ew{i+1}"], f32)[E0:E0 + EPC] for i in range(5)]
    eb = [np.asarray(inputs[f"eb{i+1}"], f32)[E0:E0 + EPC] for i in range(5)]
    km = lambda a: np.ascontiguousarray(a.transpose(1, 0, 2))
    aug = lambda w, b: np.concatenate([w, b[None]], 0)
    d = {}
    for li, pre in ((0, "e1"), (1, "e2")):
        w, b = km(ew[li]), eb[li]
        d[pre + "AA"] = w[0:128, :, 0:128].reshape(128, -1).astype(BF)
        d[pre + "AB"] = w[0:128, :, 128:196].reshape(128, -1).astype(BF)
        d[pre + "BA"] = aug(w[128:196, :, 0:128], b[:, 0:128]).reshape(69, -1).astype(BF)
        d[pre + "BB"] = aug(w[128:196, :, 128:196], b[:, 128:196]).reshape(69, -1).astype(BF)
    w3 = km(ew[2])
    d["e3A"] = w3[0:128].reshape(128, -1).astype(BF)
    d["e3B"] = aug(w3[128:196], eb[2]).reshape(69, -1).astype(BF)
    d["e4"] = aug(km(ew[3]), eb[3]).reshape(99, -1).astype(BF)
    t = np.zeros((64, 256), f32)
    for p in range(8):
        for e2 in range(2):
            e = 2 * p + e2
            rb, cb_ = 32 * e2, p * 32 + e2 * 16
            t[rb:rb + 24, cb_:cb_ + 16] = ew[4][e]
            t[rb + 24, cb_:cb_ + 16] = eb[4][e]
    d["e5p"] = t.astype(BF)

    fw1 = np.asarray(inputs["fw1"], f32)
    t = np.zeros((2, 128, 2048), f32)
    for g in range(2):
        for row in range(128):
            pp, r32 = divmod(row, 32)
            e2, o = divmod(r32, 16)
            el = (g * 4 + pp) * 2 + e2
            t[g, row, 0:FIN[1]] = fw1[(E0 + el) * 16 + o]
    d["fw1p"] = t.astype(BF)
    d["fb1d8"] = np.pad(np.asarray(inputs["fb1"], f32) / 8,
                        (0, 2048 - FIN[1])).reshape(1, 2048).astype(BF)
    fw2 = np.asarray(inputs["fw2"], f32)
    t = np.zeros((2, 128, 2048), f32)
    for kc in range(2):
        m0 = 256 * c + kc * 128
        n = max(0, min(128, FIN[1] - m0))
        if n > 0:
            t[kc, :n, 0:FIN[2]] = fw2[m0:m0 + n]
    d["fw2p"] = t.astype(BF)
    d["fb2d8"] = np.pad(np.asarray(inputs["fb2"], f32) / 8,
                        (0, 2048 - FIN[2])).reshape(1, 2048).astype(BF)
    fw3 = np.asarray(inputs["fw3"], f32)
    t = np.zeros((2, 128, 53), f32)
    for kc in range(2):
        m0 = 256 * c + kc * 128
        n = max(0, min(128, FIN[2] - m0))
        if n > 0:
            t[kc, :n] = fw3[m0:m0 + n]
    d["fw3p"] = t.astype(BF)
    d["fb3d8"] = (np.asarray(inputs["fb3"], f32) / 8).reshape(1, 53).astype(BF)
    d["onesrow"] = np.ones((1, 512), BF)
    t5f = np.zeros((64, 256), BF)
    t5f[24, :] = 1
    t5f[56, :] = 1
    d["t5fill"] = t5f
    return d


def _in_maps(inputs):
    shared = _host_shared(inputs)
    x = np.asarray(inputs["x"], np.float32)
    maps = []
    for c in range(NCORES):
        m = dict(shared)
        m.update(_host_shard(inputs, c))
        xp = np.zeros((BL, 3, 226, 226), BF)
        xp[:, :, 1:225, 1:225] = x[c * BL:(c + 1) * BL]
        m["xpad"] = np.concatenate([xp.reshape(-1), np.zeros(2, BF)])
        maps.append(m)
    return maps


def kernel(**inputs):
    global _BUILT
    if _BUILT is None:
        _BUILT = _build()
    res = run_bass_kernel_spmd(_BUILT, _in_maps(inputs), list(range(NCORES)))
    return np.sum([res.results[c]["y"] for c in range(NCORES)], axis=0,
                  dtype=np.float32)


# revision 20
# speedup vs baseline: 2.4185x; 1.2444x over previous
"""TRN2 Bass kernel for nn_CardClassifier.

CNN(4x conv3x3+relu+maxpool2) -> per-feature sigmoid attention ->
128 stacked expert MLPs -> fusion MLP (2048->2038->2028->53).

Distribution: data-parallel convs (8 cores x 4 images), then AllToAll to
expert parallelism (16 experts/core x 32 images), K-sharded fusion with
two ReduceScatters; final 53-dim partials summed on the host.

Conv engines: tap-folded K packing (im2col replicas built by strided DMA
from a host-padded input / shifted SBUF-SBUF copies), bf16 weights and
activations, fp32 PSUM. Bias+relu folded after each maxpool (commute).
"""

import sys

sys.path.insert(0, "/opt/trn_rl_repo")

import json as _json
import contextlib
import numpy as np
import ml_dtypes

import bass_rust
import concourse.bass as bass
import concourse.mybir as mybir
from concourse import tile
from concourse.bass_utils import run_bass_kernel_spmd

F32 = mybir.dt.float32
BF16 = mybir.dt.bfloat16
AF = mybir.ActivationFunctionType
ALU = None  # filled lazily
BF = ml_dtypes.bfloat16

B, H, W = 32, 224, 224
NCORES, BL = 8, 4
NF, FLAT = 128, 196
EXP_DIMS = [196, 196, 196, 98, 24, 16]
FIN = [2048, 2038, 2028, 53]
EPC = 16  # experts per core

_BUILT = None
RG = [list(range(NCORES))]


# ---------------------------------------------------------------- tilefix
def _fix_bir_json(raw: bytes) -> bytes:
    """This walrus build allows at most 1 sync-wait per instruction; Tile's
    tail drain can carry more. Split extras onto NoOp carriers."""
    d = _json.loads(raw)
    k = 0
    for fn in d.get("functions", []):
        for blk in fn.get("blocks", []):
            out = []
            for inst in blk["instructions"]:
                si = inst.get("sync_info")
                waits = (si or {}).get("on_wait") or []
                if len(waits) > 1:
                    for wchunk in waits[:-1]:
                        out.append({
                            "debug": inst.get("debug", 0),
                            "engine": inst["engine"],
                            "ins": [], "outs": [],
                            "name": f"NOPW-{k}",
                            "opcode": "NoOp",
                            "sync_info": {"on_update": [], "on_wait": [wchunk]},
                        })
                        k += 1
                    si["on_wait"] = waits[-1:]
                out.append(inst)
            blk["instructions"] = out
    return _json.dumps(d).encode()


def _vp(dims):
    return bass_rust.VecI64Pair(dims)


# ---------------------------------------------------------------- build
def _build():
    global ALU
    from concourse.alu_op_type import AluOpType as ALU_

    ALU = ALU_
    nc = bass.Bass("TRN2", target_bir_lowering=False, debug=False,
                   num_devices=NCORES)

    dp = lambda name, shape, dt: nc.declare_dram_parameter(name, list(shape), dt, isOutput=False)

    xp = dp("xpad", [BL * 3 * 226 * 226 + 2], BF16)  # +2: kx-merged rows over-read tail
    cw1p = dp("cw1p", [108, 128], BF16)
    cb1r = dp("cb1r", [128, 1], F32)
    cw2p = dp("cw2p", [96, 192], BF16)
    cb2r = dp("cb2r", [64, 1], F32)
    cw3ap = dp("cw3ap", [128, 384], BF16)
    cw3bp = dp("cw3bp", [64, 384], BF16)
    cb3r = dp("cb3r", [128, 1], F32)
    cw4p = dp("cw4p", [128, 1152], BF16)
    cb4r = dp("cb4r", [128, 1], F32)
    awr = dp("awr", [128, FLAT], BF16)
    abr = dp("abr", [128, 1], F32)

    ew_shapes = {
        "e1AA": (128, EPC * 128), "e1AB": (128, EPC * 68),
        "e1BA": (69, EPC * 128), "e1BB": (69, EPC * 68),
        "e2AA": (128, EPC * 128), "e2AB": (128, EPC * 68),
        "e2BA": (69, EPC * 128), "e2BB": (69, EPC * 68),
        "e3A": (128, EPC * 98), "e3B": (69, EPC * 98),
        "e4": (99, EPC * 24),
        "e5p": (64, 256),
    }
    ew_in = {k: dp(k, list(s), BF16) for k, s in ew_shapes.items()}

    fw1_in = dp("fw1p", [2, 128, 2048], BF16)
    fb1_in = dp("fb1d8", [1, 2048], BF16)
    fw2_in = dp("fw2p", [2, 128, 2048], BF16)
    fb2_in = dp("fb2d8", [1, 2048], BF16)
    fw3_in = dp("fw3p", [2, 128, 53], BF16)
    fb3_in = dp("fb3d8", [1, 53], BF16)
    ones_in = dp("onesrow", [1, 512], BF16)
    t5f_in = dp("t5fill", [64, 256], BF16)

    y_out = nc.declare_dram_parameter("y", [32, 53], F32, isOutput=True)

    with tile.TileContext(nc, pool_alloc_mode="queue") as tc:
        stk = contextlib.ExitStack()
        with stk:
            # ---- persistent consts
            wpool = stk.enter_context(tc.tile_pool(name="wconst", bufs=1))
            cw1 = wpool.tile([108, 128], BF16)
            nc.sync.dma_start(cw1[:], cw1p[:])
            cw2 = wpool.tile([96, 192], BF16)
            nc.sync.dma_start(cw2[:], cw2p[:])
            cw3a = wpool.tile([128, 384], BF16)
            nc.sync.dma_start(cw3a[:], cw3ap[:])
            cw3b = wpool.tile([64, 384], BF16)
            nc.sync.dma_start(cw3b[:], cw3bp[:])
            cw4 = wpool.tile([128, 1152], BF16)
            nc.sync.dma_start(cw4[:], cw4p[:])
            cb1 = wpool.tile([128, 1], F32)
            nc.sync.dma_start(cb1[:], cb1r[:])
            cb2 = wpool.tile([64, 1], F32)
            nc.sync.dma_start(cb2[:], cb2r[:])
            cb3 = wpool.tile([128, 1], F32)
            nc.sync.dma_start(cb3[:], cb3r[:])
            cb4 = wpool.tile([128, 1], F32)
            nc.sync.dma_start(cb4[:], cb4r[:])
            awsb = wpool.tile([128, FLAT], BF16)
            nc.sync.dma_start(awsb[:], awr[:])
            absb = wpool.tile([128, 1], F32)
            nc.sync.dma_start(absb[:], abr[:])
            ones1 = wpool.tile([1, 32], BF16)
            nc.vector.memset(ones1[:], 1.0)

            hpool = stk.enter_context(tc.tile_pool(name="hp", bufs=1))
            Ht = hpool.tile([128, BL * FLAT], BF16)
            HB = hpool.tile([128, BL * FLAT], BF16)

            # conv2 replica tiles (pool opened before a1r for LIFO release)
            a2pool = stk.enter_context(tc.tile_pool(name="a2r", bufs=1))
            A2Ra = [a2pool.tile([128, 56 * 58], BF16, name=f"a2a{i}")
                    for i in range(BL)]
            A2Rb = [a2pool.tile([64, 56 * 58], BF16, name=f"a2b{i}")
                    for i in range(BL)]

            # =========================================================
            # conv1: 3->32, im2col K=108 (9 taps x 4img x 3ch), M=128
            # 14 strips of 16 output rows; pool+bias+relu -> pm
            # =========================================================
            a1stk = contextlib.ExitStack()
            a1pool = a1stk.enter_context(tc.tile_pool(name="a1r", bufs=1))
            A1R = [a1pool.tile([96, 112 * 114], BF16, name=f"a1r{i}")
                   for i in range(BL)]

            with tc.tile_pool(name="c1x", bufs=2) as xpool, \
                 tc.tile_pool(name="c1v", bufs=2) as vpool, \
                 tc.tile_pool(name="c1m", bufs=2) as mpool, \
                 tc.tile_pool(name="c1pm", bufs=1) as pmpool, \
                 tc.tile_pool(name="ps1", bufs=6, space="PSUM") as psc:
                pm = pmpool.tile([128, 112 * 114], BF16)
                pmv = pm.rearrange("p (r c) -> p r c", c=114)
                nc.vector.memset(pmv[:, :, 0], 0.0)
                nc.vector.memset(pmv[:, :, 113], 0.0)
                for s in range(14):
                    r0 = 16 * s
                    X9 = xpool.tile([108, 16 * 226], BF16, tag="x9")
                    for ky in range(3):
                        src = xp[:]
                        src.ap = _vp([[1, 3], [51076, 12], [1, 3616]])
                        src.offset = src.offset + (r0 + ky) * 226
                        nc.sync.dma_start(X9[36 * ky:36 * ky + 36, :], src)
                    X9v = X9.rearrange("p (r c) -> p r c", c=226)
                    pvs = vpool.tile([128, 8 * 112], F32, tag="pvs")
                    pvsv = pvs.rearrange("p (r c) -> p r c", c=112)
                    for t in range(8):
                        P = psc.tile([128, 448], F32, tag="acc")
                        nc.tensor.matmul(P[:], cw1[:],
                                         X9v[:, 2 * t:2 * t + 2, 0:224],
                                         start=True, stop=True)
                        nc.vector.tensor_reduce(
                            pvsv[:, t:t + 1, :],
                            P.rearrange("p (r c t) -> p c r t", r=2, c=112, t=2),
                            axis=mybir.AxisListType.XY, op=ALU.max)
                    nc.vector.tensor_scalar(pmv[:, 8 * s:8 * s + 8, 1:113], pvsv[:],
                                            cb1[:, 0:1], 0.0,
                                            op0=ALU.add, op1=ALU.max)
                # replicate pm into per-image (ky,ch) layouts
                for i in range(BL):
                    av = A1R[i].rearrange("p (r c) -> p r c", c=114)
                    nc.vector.memset(av[0:32, 0, :], 0.0)
                    nc.vector.memset(av[64:96, 111, :], 0.0)
                    sp = pmv[32 * i:32 * i + 32, :, :]
                    nc.sync.dma_start(av[32:64, 0:112, :], sp)
                    nc.sync.dma_start(av[0:32, 1:112, :], sp[:, 0:111, :])
                    nc.sync.dma_start(av[64:96, 0:111, :], sp[:, 1:112, :])

            # =========================================================
            # conv2: 32->64 per img, K=96 (3ky x 32ch), 3 kx passes, M=64
            # =========================================================
            with tc.tile_pool(name="c2v", bufs=1) as v2pool, \
                 tc.tile_pool(name="c2m", bufs=1) as m2pool, \
                 tc.tile_pool(name="ps2", bufs=6, space="PSUM") as psc2:
                for i in range(BL):
                    av = A1R[i].rearrange("p (r c) -> p r c", c=114)
                    pvs2 = v2pool.tile([64, 56 * 56], F32, tag="pvs2")
                    p2v = pvs2.rearrange("p (r c) -> p r c", c=56)
                    for t in range(28):
                        P = psc2.tile([64, 448], F32, tag="acc2")
                        for kx in range(3):
                            nc.tensor.matmul(P[:], cw2[:, 64 * kx:64 * kx + 64],
                                             av[:, 4 * t:4 * t + 4, kx:kx + 112],
                                             start=(kx == 0), stop=(kx == 2))
                        nc.vector.tensor_reduce(
                            p2v[:, 2 * t:2 * t + 2, :],
                            P.rearrange("p (r a c b) -> p r c a b",
                                        r=2, a=2, c=56, b=2),
                            axis=mybir.AxisListType.XY, op=ALU.max)
                    pm2 = m2pool.tile([64, 56 * 58], BF16, tag="pm2")
                    pm2v = pm2.rearrange("p (r c) -> p r c", c=58)
                    nc.vector.memset(pm2v[:, :, 0], 0.0)
                    nc.vector.memset(pm2v[:, :, 57], 0.0)
                    nc.vector.tensor_scalar(pm2v[:, :, 1:57], pvs2[:],
                                            cb2[:, 0:1], 0.0,
                                            op0=ALU.add, op1=ALU.max)
                    aav = A2Ra[i].rearrange("p (r c) -> p r c", c=58)
                    abv = A2Rb[i].rearrange("p (r c) -> p r c", c=58)
                    nc.vector.memset(aav[0:64, 0, :], 0.0)
                    nc.vector.memset(abv[:, 55, :], 0.0)
                    nc.sync.dma_start(aav[64:128, 0:56, :], pm2v[:])
                    nc.sync.dma_start(aav[0:64, 1:56, :], pm2v[:, 0:55, :])
                    nc.sync.dma_start(abv[:, 0:55, :], pm2v[:, 1:56, :])
            a1stk.close()

            # ---- expert + fusion weights to SBUF (during conv3/4;
            # pool opened after A1R frees so the ring has room)
            ewfpool = stk.enter_context(tc.tile_pool(name="ewf", bufs=1))
            ewsb = {}
            for k, s in ew_shapes.items():
                t = ewfpool.tile(list(s), BF16, name=k + "sb")
                nc.sync.dma_start(t[:], ew_in[k][:])
                ewsb[k] = t
            fw1sb = ewfpool.tile([128, 4096], BF16)
            s1 = fw1_in[:]
            s1.ap = _vp([[2048, 128], [262144, 2], [1, 2048]])
            nc.sync.dma_start(fw1sb[:], s1)
            fw2sb = ewfpool.tile([128, 4096], BF16)
            s2 = fw2_in[:]
            s2.ap = _vp([[2048, 128], [262144, 2], [1, 2048]])
            nc.sync.dma_start(fw2sb[:], s2)
            fw3sb = ewfpool.tile([128, 106], BF16)
            s3 = fw3_in[:]
            s3.ap = _vp([[53, 128], [6784, 2], [1, 53]])
            nc.sync.dma_start(fw3sb[:], s3)
            fb1sb = ewfpool.tile([1, 2048], BF16)
            nc.sync.dma_start(fb1sb[:], fb1_in[:])
            fb2sb = ewfpool.tile([1, 2048], BF16)
            nc.sync.dma_start(fb2sb[:], fb2_in[:])
            fb3sb = ewfpool.tile([1, 53], BF16)
            nc.sync.dma_start(fb3sb[:], fb3_in[:])

            # =========================================================
            # conv3: 64->128 per img, K=128 (2ky x 64ch) + K=64 (ky2)
            # =========================================================
            a3pool = stk.enter_context(tc.tile_pool(name="a3m", bufs=1))
            A3 = [a3pool.tile([128, 30 * 30], BF16, name=f"a3_{i}")
                  for i in range(BL)]

            with tc.tile_pool(name="c3v", bufs=2) as v3pool, \
                 tc.tile_pool(name="ps3", bufs=6, space="PSUM") as psc3:
                for i in range(BL):
                    aav = A2Ra[i].rearrange("p (r c) -> p r c", c=58)
                    abv = A2Rb[i].rearrange("p (r c) -> p r c", c=58)
                    pvs3 = v3pool.tile([128, 28 * 28], F32, tag="pvs3")
                    p3v = pvs3.rearrange("p (r c) -> p r c", c=28)
                    for t in range(7):
                        P = psc3.tile([128, 448], F32, tag="acc3")
                        for kx in range(3):
                            nc.tensor.matmul(P[:], cw3a[:, 128 * kx:128 * kx + 128],
                                             aav[:, 8 * t:8 * t + 8, kx:kx + 56],
                                             start=(kx == 0), stop=False)
                            nc.tensor.matmul(P[:], cw3b[:, 128 * kx:128 * kx + 128],
                                             abv[:, 8 * t:8 * t + 8, kx:kx + 56],
                                             start=False, stop=(kx == 2))
                        nc.vector.tensor_reduce(
                            p3v[:, 4 * t:4 * t + 4, :],
                            P.rearrange("p (r a c b) -> p r c a b",
                                        r=4, a=2, c=28, b=2),
                            axis=mybir.AxisListType.XY, op=ALU.max)
                    a3v = A3[i].rearrange("p (r c) -> p r c", c=30)
                    nc.vector.memset(a3v[:, 0, :], 0.0)
                    nc.vector.memset(a3v[:, 29, :], 0.0)
                    nc.vector.memset(a3v[:, :, 0], 0.0)
                    nc.vector.memset(a3v[:, :, 29], 0.0)
                    nc.vector.tensor_scalar(a3v[:, 1:29, 1:29], p3v[:],
                                            cb3[:, 0:1], 0.0,
                                            op0=ALU.add, op1=ALU.max)

                # =====================================================
                # conv4: 128->128 per img, K=128, 9 taps via views
                # =====================================================
                for i in range(BL):
                    a3v = A3[i].rearrange("p (r c) -> p r c", c=30)
                    pvs4 = v3pool.tile([128, 14 * 14], F32, tag="pvs4")
                    p4v = pvs4.rearrange("p (r c) -> p r c", c=14)
                    for t, (rb, nr) in enumerate(((0, 16), (16, 12))):
                        P = psc3.tile([128, nr * 28], F32, tag="acc3")
                        for k in range(9):
                            ky, kx = divmod(k, 3)
                            nc.tensor.matmul(P[:], cw4[:, 128 * k:128 * k + 128],
                                             a3v[:, rb + ky:rb + ky + nr, kx:kx + 28],
                                             start=(k == 0), stop=(k == 8))
                        nc.vector.tensor_reduce(
                            p4v[:, rb // 2:rb // 2 + nr // 2, :],
                            P.rearrange("p (r a c b) -> p r c a b",
                                        r=nr // 2, a=2, c=14, b=2),
                            axis=mybir.AxisListType.XY, op=ALU.max)
                    nc.vector.tensor_scalar(Ht[:, FLAT * i:FLAT * (i + 1)], pvs4[:],
                                            cb4[:, 0:1], 0.0,
                                            op0=ALU.add, op1=ALU.max)

            # =========================================================
            # attention: att = sigmoid(feats . aw + ab); h = feats * att
            # =========================================================
            with tc.tile_pool(name="att", bufs=2) as atp:
                for i in range(BL):
                    sl = slice(FLAT * i, FLAT * (i + 1))
                    tmp = atp.tile([128, FLAT], F32, tag="tmp")
                    nc.vector.tensor_tensor(tmp[:], Ht[:, sl], awsb[:], op=ALU.mult)
                    attv = atp.tile([128, 1], F32, tag="av")
                    nc.vector.tensor_reduce(attv[:], tmp[:],
                                            axis=mybir.AxisListType.X, op=ALU.add)
                    atts = atp.tile([128, 1], F32, tag="as")
                    nc.scalar.activation(atts[:], attv[:], AF.Sigmoid, bias=absb[:])
                    nc.vector.tensor_scalar(HB[:, sl], Ht[:, sl], atts[:, 0:1],
                                            None, op0=ALU.mult)

            # =========================================================
            # AllToAll: [128e, 4i x 256fpad] -> [8s x 16e, 4i x 256fpad]
            # =========================================================
            dram = stk.enter_context(tc.tile_pool(name="dram", bufs=1, space="DRAM"))
            in_b = dram.tile([128, 1024], BF16)
            out_b = dram.tile([128, 1024], BF16)
            ibv = in_b.rearrange("p (i f) -> p i f", f=256)
            nc.sync.dma_start(ibv[:, :, 0:FLAT],
                              HB.rearrange("p (i f) -> p i f", f=FLAT)[:])
            nc.gpsimd.collective_compute(
                "AllToAll", mybir.AluOpType.bypass, replica_groups=RG,
                ins=[in_b.opt()], outs=[out_b.opt()])

            # transpose to [f, (s,e,i)] then relabel cols to (e, g=4s+i)
            tpool = stk.enter_context(tc.tile_pool(name="texp", bufs=1))
            TAraw = tpool.tile([128, 512], BF16)
            TBraw = tpool.tile([128, 512], BF16)
            for blk, dst in ((0, TAraw), (1, TBraw)):
                src = out_b[:]
                src.ap = _vp([[256, 512], [1, 128]])
                src.offset = src.offset + 128 * blk
                nc.sync.dma_start_transpose(dst[:], src)
            TA1 = tpool.tile([128, 512], BF16)
            TB1 = tpool.tile([69, 512], BF16)
            TAv = TA1.rearrange("p (e g) -> p e g", g=32)
            TBv = TB1.rearrange("p (e g) -> p e g", g=32)
            for s in range(8):
                nc.vector.tensor_copy(
                    TAv[:, :, 4 * s:4 * s + 4],
                    TAraw[:, 64 * s:64 * s + 64].rearrange("p (e i) -> p e i", i=4))
                nc.vector.tensor_copy(
                    TBv[0:68, :, 4 * s:4 * s + 4],
                    TBraw[0:68, 64 * s:64 * s + 64].rearrange("p (e i) -> p e i", i=4))
            nc.sync.dma_start(TB1[68:69, :], ones_in[:])

            # =========================================================
            # experts: 16 local experts x 32 imgs, weight-stationary
            # =========================================================
            pse = stk.enter_context(tc.tile_pool(name="pse", bufs=5, space="PSUM"))
            pse2 = stk.enter_context(tc.tile_pool(name="pse2", bufs=2, space="PSUM"))

            def elayer(TAi, TBi, pre):
                PA = pse.tile([128, 512], F32, tag="pacc")
                PB = pse.tile([68, 512], F32, tag="pacc")
                wAA, wAB = ewsb[pre + "AA"], ewsb[pre + "AB"]
                wBA, wBB = ewsb[pre + "BA"], ewsb[pre + "BB"]
                for e in range(EPC):
                    sl = slice(32 * e, 32 * e + 32)
                    nc.tensor.matmul(PA[:, sl], wAA[:, 128 * e:128 * e + 128],
                                     TAi[:, sl], start=True, stop=False)
                    nc.tensor.matmul(PA[:, sl], wBA[:, 128 * e:128 * e + 128],
                                     TBi[:, sl], start=False, stop=True)
                    nc.tensor.matmul(PB[:, sl], wAB[:, 68 * e:68 * e + 68],
                                     TAi[:, sl], start=True, stop=False)
                    nc.tensor.matmul(PB[:, sl], wBB[:, 68 * e:68 * e + 68],
                                     TBi[:, sl], start=False, stop=True)
                TAo = tpool.tile([128, 512], BF16, name=pre + "oa")
                TBo = tpool.tile([69, 512], BF16, name=pre + "ob")
                nc.scalar.activation(TAo[:], PA[:], AF.Relu)
                nc.scalar.activation(TBo[0:68, :], PB[:], AF.Relu)
                nc.sync.dma_start(TBo[68:69, :], ones_in[:])
                return TAo, TBo

            TA2, TB2 = elayer(TA1, TB1, "e1")
            TA3, TB3 = elayer(TA2, TB2, "e2")

            P98 = pse.tile([98, 512], F32, tag="pacc")
            for e in range(EPC):
                sl = slice(32 * e, 32 * e + 32)
                nc.tensor.matmul(P98[:, sl], ewsb["e3A"][:, 98 * e:98 * e + 98],
                                 TA3[:, sl], start=True, stop=False)
                nc.tensor.matmul(P98[:, sl], ewsb["e3B"][:, 98 * e:98 * e + 98],
                                 TB3[:, sl], start=False, stop=True)
            T4 = tpool.tile([99, 512], BF16)
            nc.scalar.activation(T4[0:98, :], P98[:], AF.Relu)
            nc.sync.dma_start(T4[98:99, :], ones_in[:])

            P24 = pse.tile([24, 512], F32, tag="pacc")
            for e in range(EPC):
                sl = slice(32 * e, 32 * e + 32)
                nc.tensor.matmul(P24[:, sl], ewsb["e4"][:, 24 * e:24 * e + 24],
                                 T4[:, sl], start=True, stop=True)

            # pair layout for L5: rows 0..24 even expert, 32..56 odd
            # (zeros + ones-rows prefilled from DRAM, data rows overwritten)
            T5R = tpool.tile([64, 256], BF16)
            nc.sync.dma_start(T5R[:], t5f_in[:])
            P24v = P24.rearrange("p (e g) -> p e g", g=32)
            T5Rv = T5R.rearrange("p (q g) -> p q g", g=32)
            nc.scalar.activation(T5Rv[0:24, :, :], P24v[0:24, 0:16:2, :], AF.Relu)
            nc.scalar.activation(T5Rv[32:56, :, :], P24v[0:24, 1:16:2, :], AF.Relu)

            S5 = pse2.tile([128, 64], F32, tag="ps5")
            for p in range(8):
                nc.tensor.matmul(S5[32 * (p % 4):32 * (p % 4) + 32,
                                    32 * (p // 4):32 * (p // 4) + 32],
                                 ewsb["e5p"][:, 32 * p:32 * p + 32],
                                 T5R[:, 32 * p:32 * p + 32],
                                 start=True, stop=True,
                                 tile_position=(0, 32 * (p % 4)))
            SF = tpool.tile([128, 64], BF16)
            nc.scalar.activation(SF[:], S5[:], AF.Relu)

            # =========================================================
            # fusion: K-sharded partials + ReduceScatter x2, host sum
            # =========================================================
            rs1i = dram.tile([2048, 32], BF16)
            rs1o = dram.tile([256, 32], BF16)
            rs2i = dram.tile([2048, 32], BF16)
            rs2o = dram.tile([256, 32], BF16)

            P1 = pse.tile([128, 512], F32, tag="pacc")
            for mc in range(16):
                msl = slice(32 * mc, 32 * mc + 32)
                for g in range(2):
                    nc.tensor.matmul(P1[:, msl],
                                     fw1sb[:, 2048 * g + 128 * mc:2048 * g + 128 * mc + 128],
                                     SF[:, 32 * g:32 * g + 32],
                                     start=(g == 0), stop=False)
                nc.tensor.matmul(P1[:, msl], fb1sb[:, 128 * mc:128 * mc + 128],
                                 ones1[:], start=False, stop=True)
            S1pre = tpool.tile([128, 512], BF16)
            nc.scalar.activation(S1pre[:], P1[:], AF.Copy)
            d1 = rs1i[:]
            d1.ap = _vp([[32, 128], [4096, 16], [1, 32]])
            nc.sync.dma_start(d1, S1pre[:])
            nc.gpsimd.collective_compute(
                "ReduceScatter", mybir.AluOpType.add, replica_groups=RG,
                ins=[rs1i.opt()], outs=[rs1o.opt()])
            S1c = tpool.tile([128, 64], BF16)
            sr = rs1o[:]
            sr.ap = _vp([[32, 128], [4096, 2], [1, 32]])
            nc.sync.dma_start(S1c[:], sr)
            S1 = tpool.tile([128, 64], BF16)
            nc.scalar.activation(S1[:], S1c[:], AF.Relu)

            P2 = pse.tile([128, 512], F32, tag="pacc")
            for mc in range(16):
                msl = slice(32 * mc, 32 * mc + 32)
                for kc in range(2):
                    nc.tensor.matmul(P2[:, msl],
                                     fw2sb[:, 2048 * kc + 128 * mc:2048 * kc + 128 * mc + 128],
                                     S1[:, 32 * kc:32 * kc + 32],
                                     start=(kc == 0), stop=False)
                nc.tensor.matmul(P2[:, msl], fb2sb[:, 128 * mc:128 * mc + 128],
                                 ones1[:], start=False, stop=True)
            S2pre = tpool.tile([128, 512], BF16)
            nc.scalar.activation(S2pre[:], P2[:], AF.Copy)
            d2 = rs2i[:]
            d2.ap = _vp([[32, 128], [4096, 16], [1, 32]])
            nc.sync.dma_start(d2, S2pre[:])
            nc.gpsimd.collective_compute(
                "ReduceScatter", mybir.AluOpType.add, replica_groups=RG,
                ins=[rs2i.opt()], outs=[rs2o.opt()])
            S2c = tpool.tile([128, 64], BF16)
            sr2 = rs2o[:]
            sr2.ap = _vp([[32, 128], [4096, 2], [1, 32]])
            nc.sync.dma_start(S2c[:], sr2)
            S2 = tpool.tile([128, 64], BF16)
            nc.scalar.activation(S2[:], S2c[:], AF.Relu)

            P3 = pse2.tile([53, 32], F32, tag="ps5")
            for kc in range(2):
                nc.tensor.matmul(P3[:], fw3sb[:, 53 * kc:53 * kc + 53],
                                 S2[:, 32 * kc:32 * kc + 32],
                                 start=(kc == 0), stop=False)
            nc.tensor.matmul(P3[:], fb3sb[:], ones1[:], start=False, stop=True)
            S3 = tpool.tile([53, 32], F32)
            nc.scalar.activation(S3[:], P3[:], AF.Copy)
            nc.sync.dma_start(y_out[:].rearrange("b o -> o b"), S3[:])

    orig = nc.to_json_bytes
    nc.to_json_bytes = lambda: _fix_bir_json(orig())
    return nc


# ---------------------------------------------------------------- host prep
def _host_shared(inputs):
    f32 = np.float32
    cw = [np.asarray(inputs[f"cw{i+1}"], f32) for i in range(4)]
    cb = [np.asarray(inputs[f"cb{i+1}"], f32) for i in range(4)]
    d = {}
    t = np.zeros((108, 128), f32)
    for ky in range(3):
        for kx in range(3):
            blk = cw[0][:, :, ky, kx].T
            for img in range(4):
                r = (ky * 3 + kx) * 12 + img * 3
                t[r:r + 3, img * 32:(img + 1) * 32] = blk
    d["cw1p"] = t.astype(BF)
    d["cb1r"] = np.tile(cb[0], 4).reshape(128, 1)
    t = np.zeros((96, 192), f32)
    for ky in range(3):
        for kx in range(3):
            t[ky * 32:(ky + 1) * 32, kx * 64:(kx + 1) * 64] = cw[1][:, :, ky, kx].T
    d["cw2p"] = t.astype(BF)
    d["cb2r"] = cb[1].reshape(64, 1)
    ta = np.zeros((128, 384), f32)
    tb = np.zeros((64, 384), f32)
    for kx in range(3):
        for ky in range(2):
            ta[ky * 64:(ky + 1) * 64, kx * 128:(kx + 1) * 128] = cw[2][:, :, ky, kx].T
        tb[:, kx * 128:(kx + 1) * 128] = cw[2][:, :, 2, kx].T
    d["cw3ap"] = ta.astype(BF)
    d["cw3bp"] = tb.astype(BF)
    d["cb3r"] = cb[2].reshape(128, 1)
    t = np.zeros((128, 1152), f32)
    for k in range(9):
        ky, kx = divmod(k, 3)
        t[:, k * 128:(k + 1) * 128] = cw[3][:, :, ky, kx].T
    d["cw4p"] = t.astype(BF)
    d["cb4r"] = cb[3].reshape(128, 1)
    d["awr"] = np.asarray(inputs["aw"], f32)[:, :, 0].astype(BF)
    d["abr"] = np.asarray(inputs["ab"], f32).reshape(128, 1)
    return d


def _host_shard(inputs, c):
    f32 = np.float32
    E0 = EPC * c
    ew = [np.asarray(inputs[f"ew{i+1}"], f32)[E0:E0 + EPC] for i in range(5)]
    eb = [np.asarray(inputs[f"eb{i+1}"], f32)[E0:E0 + EPC] for i in range(5)]
    km = lambda a: np.ascontiguousarray(a.transpose(1, 0, 2))
    aug = lambda w, b: np.concatenate([w, b[None]], 0)
    d = {}
    for li, pre in ((0, "e1"), (1, "e2")):
        w, b = km(ew[li]), eb[li]
        d[pre + "AA"] = w[0:128, :, 0:128].reshape(128, -1).astype(BF)
        d[pre + "AB"] = w[0:128, :, 128:196].reshape(128, -1).astype(BF)
        d[pre + "BA"] = aug(w[128:196, :, 0:128], b[:, 0:128]).reshape(69, -1).astype(BF)
        d[pre + "BB"] = aug(w[128:196, :, 128:196], b[:, 128:196]).reshape(69, -1).astype(BF)
    w3 = km(ew[2])
    d["e3A"] = w3[0:128].reshape(128, -1).astype(BF)
    d["e3B"] = aug(w3[128:196], eb[2]).reshape(69, -1).astype(BF)
    d["e4"] = aug(km(ew[3]), eb[3]).reshape(99, -1).astype(BF)
    t = np.zeros((64, 256), f32)
    for p in range(8):
        for e2 in range(2):
            e = 2 * p + e2
            rb, cb_ = 32 * e2, p * 32 + e2 * 16
            t[rb:rb + 24, cb_:cb_ + 16] = ew[4][e]
            t[rb + 24, cb_:cb_ + 16] = eb[4][e]
    d["e5p"] = t.astype(BF)

    fw1 = np.asarray(inputs["fw1"], f32)
    t = np.zeros((2, 128, 2048), f32)
    for g in range(2):
        for row in range(128):
            pp, r32 = divmod(row, 32)
            e2, o = divmod(r32, 16)
            el = (g * 4 + pp) * 2 + e2
            t[g, row, 0:FIN[1]] = fw1[(E0 + el) * 16 + o]
    d["fw1p"] = t.astype(BF)
    d["fb1d8"] = np.pad(np.asarray(inputs["fb1"], f32) / 8,
                        (0, 2048 - FIN[1])).reshape(1, 2048).astype(BF)
    fw2 = np.asarray(inputs["fw2"], f32)
    t = np.zeros((2, 128, 2048), f32)
    for kc in range(2):
        m0 = 256 * c + kc * 128
        n = max(0, min(128, FIN[1] - m0))
        if n > 0:
            t[kc, :n, 0:FIN[2]] = fw2[m0:m0 + n]
    d["fw2p"] = t.astype(BF)
    d["fb2d8"] = np.pad(np.asarray(inputs["fb2"], f32) / 8,
                        (0, 2048 - FIN[2])).reshape(1, 2048).astype(BF)
    fw3 = np.asarray(inputs["fw3"], f32)
    t = np.zeros((2, 128, 53), f32)
    for kc in range(2):
        m0 = 256 * c + kc * 128
        n = max(0, min(128, FIN[2] - m0))
        if n > 0:
            t[kc, :n] = fw3[m0:m0 + n]
    d["fw3p"] = t.astype(BF)
    d["fb3d8"] = (np.asarray(inputs["fb3"], f32) / 8).reshape(1, 53).astype(BF)
    d["onesrow"] = np.ones((1, 512), BF)
    t5f = np.zeros((64, 256), BF)
    t5f[24, :] = 1
    t5f[56, :] = 1
    d["t5fill"] = t5f
    return d


def _in_maps(inputs):
    shared = _host_shared(inputs)
    x = np.asarray(inputs["x"], np.float32)
    maps = []
    for c in range(NCORES):
        m = dict(shared)
        m.update(_host_shard(inputs, c))
        xp = np.zeros((BL, 3, 226, 226), BF)
        xp[:, :, 1:225, 1:225] = x[c * BL:(c + 1) * BL]
        m["xpad"] = np.concatenate([xp.reshape(-1), np.zeros(2, BF)])
        maps.append(m)
    return maps


def kernel(**inputs):
    global _BUILT
    if _BUILT is None:
        _BUILT = _build()
    res = run_bass_kernel_spmd(_BUILT, _in_maps(inputs), list(range(NCORES)))
    return np.sum([res.results[c]["y"] for c in range(NCORES)], axis=0,
                  dtype=np.float32)


# revision 24
# speedup vs baseline: 2.5394x; 1.0500x over previous
"""TRN2 Bass kernel for nn_CardClassifier.

CNN(4x conv3x3+relu+maxpool2) -> per-feature sigmoid attention ->
128 stacked expert MLPs -> fusion MLP (2048->2038->2028->53).

Distribution: data-parallel convs (8 cores x 4 images), then AllToAll to
expert parallelism (16 experts/core x 32 images), K-sharded fusion with
two ReduceScatters; final 53-dim partials summed on the host.

Conv engines: tap-folded K packing (im2col replicas built by strided DMA
from a host-padded input / shifted SBUF-SBUF copies), bf16 weights and
activations, fp32 PSUM. Bias+relu folded after each maxpool (commute).
"""

import sys

sys.path.insert(0, "/opt/trn_rl_repo")

import json as _json
import contextlib
import numpy as np
import ml_dtypes

import bass_rust
import concourse.bass as bass
import concourse.mybir as mybir
from concourse import tile
from concourse.bass_utils import run_bass_kernel_spmd

F32 = mybir.dt.float32
BF16 = mybir.dt.bfloat16
AF = mybir.ActivationFunctionType
ALU = None  # filled lazily
BF = ml_dtypes.bfloat16

B, H, W = 32, 224, 224
NCORES, BL = 8, 4
NF, FLAT = 128, 196
EXP_DIMS = [196, 196, 196, 98, 24, 16]
FIN = [2048, 2038, 2028, 53]
EPC = 16  # experts per core

_BUILT = None
RG = [list(range(NCORES))]


# ---------------------------------------------------------------- tilefix
def _fix_bir_json(raw: bytes) -> bytes:
    """This walrus build allows at most 1 sync-wait per instruction; Tile's
    tail drain can carry more. Split extras onto NoOp carriers."""
    d = _json.loads(raw)
    k = 0
    for fn in d.get("functions", []):
        for blk in fn.get("blocks", []):
            out = []
            for inst in blk["instructions"]:
                si = inst.get("sync_info")
                waits = (si or {}).get("on_wait") or []
                if len(waits) > 1:
                    for wchunk in waits[:-1]:
                        out.append({
                            "debug": inst.get("debug", 0),
                            "engine": inst["engine"],
                            "ins": [], "outs": [],
                            "name": f"NOPW-{k}",
                            "opcode": "NoOp",
                            "sync_info": {"on_update": [], "on_wait": [wchunk]},
                        })
                        k += 1
                    si["on_wait"] = waits[-1:]
                out.append(inst)
            blk["instructions"] = out
    return _json.dumps(d).encode()


def _vp(dims):
    return bass_rust.VecI64Pair(dims)


# ---------------------------------------------------------------- build
def _build():
    global ALU
    from concourse.alu_op_type import AluOpType as ALU_

    ALU = ALU_
    nc = bass.Bass("TRN2", target_bir_lowering=False, debug=False,
                   num_devices=NCORES)

    dp = lambda name, shape, dt: nc.declare_dram_parameter(name, list(shape), dt, isOutput=False)

    xp = dp("xpad", [BL * 3 * 226 * 226 + 2], BF16)  # +2: kx-merged rows over-read tail
    cw1p = dp("cw1p", [108, 128], BF16)
    cb1r = dp("cb1r", [128, 1], F32)
    cw2p = dp("cw2p", [96, 192], BF16)
    cb2r = dp("cb2r", [64, 1], F32)
    cw3ap = dp("cw3ap", [128, 384], BF16)
    cw3bp = dp("cw3bp", [64, 384], BF16)
    cb3r = dp("cb3r", [128, 1], F32)
    cw4p = dp("cw4p", [128, 1152], BF16)
    cb4r = dp("cb4r", [128, 1], F32)
    awr = dp("awr", [128, FLAT], BF16)
    abr = dp("abr", [128, 1], F32)

    ew_shapes = {
        "e1AA": (128, EPC * 128), "e1AB": (128, EPC * 68),
        "e1BA": (69, EPC * 128), "e1BB": (69, EPC * 68),
        "e2AA": (128, EPC * 128), "e2AB": (128, EPC * 68),
        "e2BA": (69, EPC * 128), "e2BB": (69, EPC * 68),
        "e3A": (128, EPC * 98), "e3B": (69, EPC * 98),
        "e4": (99, EPC * 24),
        "e5p": (64, 256),
    }
    ew_in = {k: dp(k, list(s), BF16) for k, s in ew_shapes.items()}

    fw1_in = dp("fw1p", [2, 128, 2048], BF16)
    fb1_in = dp("fb1d8", [1, 2048], BF16)
    fw2_in = dp("fw2p", [2, 128, 2048], BF16)
    fb2_in = dp("fb2d8", [1, 2048], BF16)
    fw3_in = dp("fw3p", [2, 128, 53], BF16)
    fb3_in = dp("fb3d8", [1, 53], BF16)
    ones_in = dp("onesrow", [1, 512], BF16)
    t5f_in = dp("t5fill", [64, 256], BF16)

    y_out = nc.declare_dram_parameter("y", [32, 53], F32, isOutput=True)

    with tile.TileContext(nc, pool_alloc_mode="queue") as tc:
        stk = contextlib.ExitStack()
        with stk:
            # ---- persistent consts
            wpool = stk.enter_context(tc.tile_pool(name="wconst", bufs=1))
            cw1 = wpool.tile([108, 128], BF16)
            nc.sync.dma_start(cw1[:], cw1p[:])
            cw2 = wpool.tile([96, 192], BF16)
            nc.sync.dma_start(cw2[:], cw2p[:])
            cw3a = wpool.tile([128, 384], BF16)
            nc.sync.dma_start(cw3a[:], cw3ap[:])
            cw3b = wpool.tile([64, 384], BF16)
            nc.sync.dma_start(cw3b[:], cw3bp[:])
            cw4 = wpool.tile([128, 1152], BF16)
            nc.sync.dma_start(cw4[:], cw4p[:])
            cb1 = wpool.tile([128, 1], F32)
            nc.sync.dma_start(cb1[:], cb1r[:])
            cb2 = wpool.tile([64, 1], F32)
            nc.sync.dma_start(cb2[:], cb2r[:])
            cb3 = wpool.tile([128, 1], F32)
            nc.sync.dma_start(cb3[:], cb3r[:])
            cb4 = wpool.tile([128, 1], F32)
            nc.sync.dma_start(cb4[:], cb4r[:])
            awsb = wpool.tile([128, FLAT], BF16)
            nc.sync.dma_start(awsb[:], awr[:])
            absb = wpool.tile([128, 1], F32)
            nc.sync.dma_start(absb[:], abr[:])
            ones1 = wpool.tile([1, 32], BF16)
            nc.vector.memset(ones1[:], 1.0)

            hpool = stk.enter_context(tc.tile_pool(name="hp", bufs=1))
            Ht = [hpool.tile([128, FLAT], BF16, name=f"ht{i}") for i in range(BL)]
            HB = [hpool.tile([128, FLAT], BF16, name=f"hb{i}") for i in range(BL)]

            # conv2 replica tiles (pool opened before a1r for LIFO release)
            a2pool = stk.enter_context(tc.tile_pool(name="a2r", bufs=1))
            A2Ra = [a2pool.tile([128, 56 * 58], BF16, name=f"a2a{i}")
                    for i in range(BL)]
            A2Rb = [a2pool.tile([64, 56 * 58], BF16, name=f"a2b{i}")
                    for i in range(BL)]

            # =========================================================
            # conv1: 3->32, im2col K=108 (9 taps x 4img x 3ch), M=128
            # 14 strips of 16 output rows; pool+bias+relu -> pm
            # =========================================================
            a1stk = contextlib.ExitStack()
            a1pool = a1stk.enter_context(tc.tile_pool(name="a1r", bufs=1))
            A1R = [a1pool.tile([96, 112 * 114], BF16, name=f"a1r{i}")
                   for i in range(BL)]

            with tc.tile_pool(name="c1x", bufs=2) as xpool, \
                 tc.tile_pool(name="c1v", bufs=2) as vpool, \
                 tc.tile_pool(name="c1m", bufs=2) as mpool, \
                 tc.tile_pool(name="c1pm", bufs=1) as pmpool, \
                 tc.tile_pool(name="ps1", bufs=6, space="PSUM") as psc:
                pm = pmpool.tile([128, 112 * 114], BF16)
                pmv = pm.rearrange("p (r c) -> p r c", c=114)
                nc.vector.memset(pmv[:, :, 0], 0.0)
                nc.vector.memset(pmv[:, :, 113], 0.0)
                for s in range(14):
                    r0 = 16 * s
                    X9 = xpool.tile([108, 16 * 226], BF16, tag="x9")
                    for ky, eng in ((0, nc.sync), (1, nc.scalar), (2, nc.gpsimd)):
                        src = xp[:]
                        src.ap = _vp([[1, 3], [51076, 12], [1, 3616]])
                        src.offset = src.offset + (r0 + ky) * 226
                        eng.dma_start(X9[36 * ky:36 * ky + 36, :], src)
                    X9v = X9.rearrange("p (r c) -> p r c", c=226)
                    pvs = vpool.tile([128, 8 * 112], F32, tag="pvs")
                    pvsv = pvs.rearrange("p (r c) -> p r c", c=112)
                    for t in range(8):
                        P = psc.tile([128, 448], F32, tag="acc")
                        nc.tensor.matmul(P[:], cw1[:],
                                         X9v[:, 2 * t:2 * t + 2, 0:224],
                                         start=True, stop=True)
                        nc.vector.tensor_reduce(
                            pvsv[:, t:t + 1, :],
                            P.rearrange("p (r c t) -> p c r t", r=2, c=112, t=2),
                            axis=mybir.AxisListType.XY, op=ALU.max)
                    nc.vector.tensor_scalar(pmv[:, 8 * s:8 * s + 8, 1:113], pvsv[:],
                                            cb1[:, 0:1], 0.0,
                                            op0=ALU.add, op1=ALU.max)
                # replicate pm into per-image (ky,ch) layouts
                for i in range(BL):
                    av = A1R[i].rearrange("p (r c) -> p r c", c=114)
                    nc.vector.memset(av[0:32, 0, :], 0.0)
                    nc.vector.memset(av[64:96, 111, :], 0.0)
                    sp = pmv[32 * i:32 * i + 32, :, :]
                    nc.sync.dma_start(av[32:64, 0:112, :], sp)
                    nc.scalar.dma_start(av[0:32, 1:112, :], sp[:, 0:111, :])
                    nc.gpsimd.dma_start(av[64:96, 0:111, :], sp[:, 1:112, :])

            # =========================================================
            # conv2: 32->64 per img, K=96 (3ky x 32ch), 3 kx passes, M=64
            # =========================================================
            with tc.tile_pool(name="c2v", bufs=1) as v2pool, \
                 tc.tile_pool(name="c2m", bufs=1) as m2pool, \
                 tc.tile_pool(name="ps2", bufs=6, space="PSUM") as psc2:
                for i in range(BL):
                    av = A1R[i].rearrange("p (r c) -> p r c", c=114)
                    pvs2 = v2pool.tile([64, 56 * 56], F32, tag="pvs2")
                    p2v = pvs2.rearrange("p (r c) -> p r c", c=56)
                    for t in range(28):
                        P = psc2.tile([64, 448], F32, tag="acc2")
                        for kx in range(3):
                            nc.tensor.matmul(P[:], cw2[:, 64 * kx:64 * kx + 64],
                                             av[:, 4 * t:4 * t + 4, kx:kx + 112],
                                             start=(kx == 0), stop=(kx == 2))
                        nc.vector.tensor_reduce(
                            p2v[:, 2 * t:2 * t + 2, :],
                            P.rearrange("p (r a c b) -> p r c a b",
                                        r=2, a=2, c=56, b=2),
                            axis=mybir.AxisListType.XY, op=ALU.max)
                    pm2 = m2pool.tile([64, 56 * 58], BF16, tag="pm2")
                    pm2v = pm2.rearrange("p (r c) -> p r c", c=58)
                    nc.vector.memset(pm2v[:, :, 0], 0.0)
                    nc.vector.memset(pm2v[:, :, 57], 0.0)
                    nc.vector.tensor_scalar(pm2v[:, :, 1:57], pvs2[:],
                                            cb2[:, 0:1], 0.0,
                                            op0=ALU.add, op1=ALU.max)
                    aav = A2Ra[i].rearrange("p (r c) -> p r c", c=58)
                    abv = A2Rb[i].rearrange("p (r c) -> p r c", c=58)
                    nc.vector.memset(aav[0:64, 0, :], 0.0)
                    nc.vector.memset(abv[:, 55, :], 0.0)
                    nc.sync.dma_start(aav[64:128, 0:56, :], pm2v[:])
                    nc.scalar.dma_start(aav[0:64, 1:56, :], pm2v[:, 0:55, :])
                    nc.gpsimd.dma_start(abv[:, 0:55, :], pm2v[:, 1:56, :])
            a1stk.close()

            # ---- expert + fusion weights to SBUF (during conv3/4;
            # pool opened after A1R frees so the ring has room)
            ewfpool = stk.enter_context(tc.tile_pool(name="ewf", bufs=1))
            ewsb = {}
            for k, s in ew_shapes.items():
                t = ewfpool.tile(list(s), BF16, name=k + "sb")
                nc.gpsimd.dma_start(t[:], ew_in[k][:])
                ewsb[k] = t
            fw1sb = ewfpool.tile([128, 4096], BF16)
            s1 = fw1_in[:]
            s1.ap = _vp([[2048, 128], [262144, 2], [1, 2048]])
            nc.gpsimd.dma_start(fw1sb[:], s1)
            fw2sb = ewfpool.tile([128, 4096], BF16)
            s2 = fw2_in[:]
            s2.ap = _vp([[2048, 128], [262144, 2], [1, 2048]])
            nc.gpsimd.dma_start(fw2sb[:], s2)
            fw3sb = ewfpool.tile([128, 106], BF16)
            s3 = fw3_in[:]
            s3.ap = _vp([[53, 128], [6784, 2], [1, 53]])
            nc.gpsimd.dma_start(fw3sb[:], s3)
            fb1sb = ewfpool.tile([1, 2048], BF16)
            nc.gpsimd.dma_start(fb1sb[:], fb1_in[:])
            fb2sb = ewfpool.tile([1, 2048], BF16)
            nc.gpsimd.dma_start(fb2sb[:], fb2_in[:])
            fb3sb = ewfpool.tile([1, 53], BF16)
            nc.gpsimd.dma_start(fb3sb[:], fb3_in[:])

            # =========================================================
            # conv3: 64->128 per img, K=128 (2ky x 64ch) + K=64 (ky2)
            # =========================================================
            a3pool = stk.enter_context(tc.tile_pool(name="a3m", bufs=1))
            A3 = [a3pool.tile([128, 30 * 30], BF16, name=f"a3_{i}")
                  for i in range(BL)]

            with tc.tile_pool(name="c3v", bufs=2) as v3pool, \
                 tc.tile_pool(name="ps3", bufs=6, space="PSUM") as psc3:
                aavs = [A2Ra[i].rearrange("p (r c) -> p r c", c=58) for i in range(BL)]
                abvs = [A2Rb[i].rearrange("p (r c) -> p r c", c=58) for i in range(BL)]
                pvs3s = [v3pool.tile([128, 28 * 28], F32, tag=f"pvs3_{i}", name=f"pvs3_{i}")
                         for i in range(BL)]
                p3vs = [t.rearrange("p (r c) -> p r c", c=28) for t in pvs3s]
                for t in range(7):
                    Ps = [psc3.tile([128, 448], F32, tag="acc3", name=f"p3t{t}_{j}") for j in range(BL)]
                    for kx in range(3):
                        for i in range(BL):
                            nc.tensor.matmul(Ps[i][:], cw3a[:, 128 * kx:128 * kx + 128],
                                             aavs[i][:, 8 * t:8 * t + 8, kx:kx + 56],
                                             start=(kx == 0), stop=False)
                        for i in range(BL):
                            nc.tensor.matmul(Ps[i][:], cw3b[:, 128 * kx:128 * kx + 128],
                                             abvs[i][:, 8 * t:8 * t + 8, kx:kx + 56],
                                             start=False, stop=(kx == 2))
                    for i in range(BL):
                        nc.vector.tensor_reduce(
                            p3vs[i][:, 4 * t:4 * t + 4, :],
                            Ps[i].rearrange("p (r a c b) -> p r c a b",
                                            r=4, a=2, c=28, b=2),
                            axis=mybir.AxisListType.XY, op=ALU.max)
                for i in range(BL):
                    a3v = A3[i].rearrange("p (r c) -> p r c", c=30)
                    nc.vector.memset(a3v[:, 0, :], 0.0)
                    nc.vector.memset(a3v[:, 29, :], 0.0)
                    nc.vector.memset(a3v[:, :, 0], 0.0)
                    nc.vector.memset(a3v[:, :, 29], 0.0)
                    nc.vector.tensor_scalar(a3v[:, 1:29, 1:29], pvs3s[i][:],
                                            cb3[:, 0:1], 0.0,
                                            op0=ALU.add, op1=ALU.max)

                # =====================================================
                # conv4: 128->128 per img, K=128, 9 taps via views
                # =====================================================
                a3vs = [A3[i].rearrange("p (r c) -> p r c", c=30) for i in range(BL)]
                pvs4s = [v3pool.tile([128, 14 * 14], F32, tag=f"pvs4_{i}", name=f"pvs4_{i}")
                         for i in range(BL)]
                p4vs = [t.rearrange("p (r c) -> p r c", c=14) for t in pvs4s]
                for t, (rb, nr) in enumerate(((0, 16), (16, 12))):
                    Ps = [psc3.tile([128, nr * 28], F32, tag="acc3", name=f"p4t{t}_{j}") for j in range(BL)]
                    for k in range(9):
                        ky, kx = divmod(k, 3)
                        for i in range(BL):
                            nc.tensor.matmul(Ps[i][:], cw4[:, 128 * k:128 * k + 128],
                                             a3vs[i][:, rb + ky:rb + ky + nr, kx:kx + 28],
                                             start=(k == 0), stop=(k == 8))
                    for i in range(BL):
                        nc.vector.tensor_reduce(
                            p4vs[i][:, rb // 2:rb // 2 + nr // 2, :],
                            Ps[i].rearrange("p (r a c b) -> p r c a b",
                                            r=nr // 2, a=2, c=14, b=2),
                            axis=mybir.AxisListType.XY, op=ALU.max)
                for i in range(BL):
                    nc.vector.tensor_scalar(Ht[i][:], pvs4s[i][:],
                                            cb4[:, 0:1], 0.0,
                                            op0=ALU.add, op1=ALU.max)

            # =========================================================
            # attention: att = sigmoid(feats . aw + ab); h = feats * att
            # =========================================================
            with tc.tile_pool(name="att", bufs=2) as atp:
                for i in range(BL):
                    tmp = atp.tile([128, FLAT], F32, tag="tmp")
                    nc.vector.tensor_tensor(tmp[:], Ht[i][:], awsb[:], op=ALU.mult)
                    attv = atp.tile([128, 1], F32, tag="av")
                    nc.vector.tensor_reduce(attv[:], tmp[:],
                                            axis=mybir.AxisListType.X, op=ALU.add)
                    atts = atp.tile([128, 1], F32, tag="as")
                    nc.scalar.activation(atts[:], attv[:], AF.Sigmoid, bias=absb[:])
                    nc.vector.tensor_scalar(HB[i][:], Ht[i][:], atts[:, 0:1],
                                            None, op0=ALU.mult)

            # =========================================================
            # AllToAll: [128e, 4i x 256fpad] -> [8s x 16e, 4i x 256fpad]
            # =========================================================
            dram = stk.enter_context(tc.tile_pool(name="dram", bufs=1, space="DRAM"))
            in_b = dram.tile([128, 1024], BF16)
            out_b = dram.tile([128, 1024], BF16)
            ibv = in_b.rearrange("p (i f) -> p i f", f=256)
            for i in range(BL):
                nc.sync.dma_start(ibv[:, i, 0:FLAT], HB[i][:])
            nc.gpsimd.collective_compute(
                "AllToAll", mybir.AluOpType.bypass, replica_groups=RG,
                ins=[in_b.opt()], outs=[out_b.opt()])

            # transpose to [f, (s,e,i)] then relabel cols to (e, g=4s+i)
            tpool = stk.enter_context(tc.tile_pool(name="texp", bufs=1))
            TAraw = tpool.tile([128, 512], BF16)
            TBraw = tpool.tile([128, 512], BF16)
            for blk, dst in ((0, TAraw), (1, TBraw)):
                src = out_b[:]
                src.ap = _vp([[256, 512], [1, 128]])
                src.offset = src.offset + 128 * blk
                nc.sync.dma_start_transpose(dst[:], src)
            TA1 = tpool.tile([128, 512], BF16)
            TB1 = tpool.tile([69, 512], BF16)
            TAv = TA1.rearrange("p (e g) -> p e g", g=32)
            TBv = TB1.rearrange("p (e g) -> p e g", g=32)
            for s in range(8):
                nc.vector.tensor_copy(
                    TAv[:, :, 4 * s:4 * s + 4],
                    TAraw[:, 64 * s:64 * s + 64].rearrange("p (e i) -> p e i", i=4))
                nc.vector.tensor_copy(
                    TBv[0:68, :, 4 * s:4 * s + 4],
                    TBraw[0:68, 64 * s:64 * s + 64].rearrange("p (e i) -> p e i", i=4))
            nc.sync.dma_start(TB1[68:69, :], ones_in[:])

            # =========================================================
            # experts: 16 local experts x 32 imgs, weight-stationary
            # =========================================================
            pse = stk.enter_context(tc.tile_pool(name="pse", bufs=5, space="PSUM"))
            pse2 = stk.enter_context(tc.tile_pool(name="pse2", bufs=2, space="PSUM"))

            def elayer(TAi, TBi, pre):
                PA = pse.tile([128, 512], F32, tag="pacc")
                PB = pse.tile([68, 512], F32, tag="pacc")
                wAA, wAB = ewsb[pre + "AA"], ewsb[pre + "AB"]
                wBA, wBB = ewsb[pre + "BA"], ewsb[pre + "BB"]
                for e in range(EPC):
                    sl = slice(32 * e, 32 * e + 32)
                    nc.tensor.matmul(PA[:, sl], wAA[:, 128 * e:128 * e + 128],
                                     TAi[:, sl], start=True, stop=False)
                    nc.tensor.matmul(PA[:, sl], wBA[:, 128 * e:128 * e + 128],
                                     TBi[:, sl], start=False, stop=True)
                    nc.tensor.matmul(PB[:, sl], wAB[:, 68 * e:68 * e + 68],
                                     TAi[:, sl], start=True, stop=False)
                    nc.tensor.matmul(PB[:, sl], wBB[:, 68 * e:68 * e + 68],
                                     TBi[:, sl], start=False, stop=True)
                TAo = tpool.tile([128, 512], BF16, name=pre + "oa")
                TBo = tpool.tile([69, 512], BF16, name=pre + "ob")
                nc.scalar.activation(TAo[:], PA[:], AF.Relu)
                nc.scalar.activation(TBo[0:68, :], PB[:], AF.Relu)
                nc.sync.dma_start(TBo[68:69, :], ones_in[:])
                return TAo, TBo

            TA2, TB2 = elayer(TA1, TB1, "e1")
            TA3, TB3 = elayer(TA2, TB2, "e2")

            P98 = pse.tile([98, 512], F32, tag="pacc")
            for e in range(EPC):
                sl = slice(32 * e, 32 * e + 32)
                nc.tensor.matmul(P98[:, sl], ewsb["e3A"][:, 98 * e:98 * e + 98],
                                 TA3[:, sl], start=True, stop=False)
                nc.tensor.matmul(P98[:, sl], ewsb["e3B"][:, 98 * e:98 * e + 98],
                                 TB3[:, sl], start=False, stop=True)
            T4 = tpool.tile([99, 512], BF16)
            nc.scalar.activation(T4[0:98, :], P98[:], AF.Relu)
            nc.sync.dma_start(T4[98:99, :], ones_in[:])

            P24 = pse.tile([24, 512], F32, tag="pacc")
            for e in range(EPC):
                sl = slice(32 * e, 32 * e + 32)
                nc.tensor.matmul(P24[:, sl], ewsb["e4"][:, 24 * e:24 * e + 24],
                                 T4[:, sl], start=True, stop=True)

            # pair layout for L5: rows 0..24 even expert, 32..56 odd
            # (zeros + ones-rows prefilled from DRAM, data rows overwritten)
            T5R = tpool.tile([64, 256], BF16)
            nc.sync.dma_start(T5R[:], t5f_in[:])
            P24v = P24.rearrange("p (e g) -> p e g", g=32)
            T5Rv = T5R.rearrange("p (q g) -> p q g", g=32)
            nc.scalar.activation(T5Rv[0:24, :, :], P24v[0:24, 0:16:2, :], AF.Relu)
            nc.scalar.activation(T5Rv[32:56, :, :], P24v[0:24, 1:16:2, :], AF.Relu)

            S5 = pse2.tile([128, 64], F32, tag="ps5")
            for p in range(8):
                nc.tensor.matmul(S5[32 * (p % 4):32 * (p % 4) + 32,
                                    32 * (p // 4):32 * (p // 4) + 32],
                                 ewsb["e5p"][:, 32 * p:32 * p + 32],
                                 T5R[:, 32 * p:32 * p + 32],
                                 start=True, stop=True,
                                 tile_position=(0, 32 * (p % 4)))
            SF = tpool.tile([128, 64], BF16)
            nc.scalar.activation(SF[:], S5[:], AF.Relu)

            # =========================================================
            # fusion: K-sharded partials + ReduceScatter x2, host sum
            # =========================================================
            rs1i = dram.tile([2048, 32], BF16)
            rs1o = dram.tile([256, 32], BF16)
            rs2i = dram.tile([2048, 32], BF16)
            rs2o = dram.tile([256, 32], BF16)

            P1 = pse.tile([128, 512], F32, tag="pacc")
            for mc in range(16):
                msl = slice(32 * mc, 32 * mc + 32)
                for g in range(2):
                    nc.tensor.matmul(P1[:, msl],
                                     fw1sb[:, 2048 * g + 128 * mc:2048 * g + 128 * mc + 128],
                                     SF[:, 32 * g:32 * g + 32],
                                     start=(g == 0), stop=False)
                nc.tensor.matmul(P1[:, msl], fb1sb[:, 128 * mc:128 * mc + 128],
                                 ones1[:], start=False, stop=True)
            S1pre = tpool.tile([128, 512], BF16)
            nc.scalar.activation(S1pre[:], P1[:], AF.Copy)
            d1 = rs1i[:]
            d1.ap = _vp([[32, 128], [4096, 16], [1, 32]])
            nc.sync.dma_start(d1, S1pre[:])
            nc.gpsimd.collective_compute(
                "ReduceScatter", mybir.AluOpType.add, replica_groups=RG,
                ins=[rs1i.opt()], outs=[rs1o.opt()])
            S1c = tpool.tile([128, 64], BF16)
            sr = rs1o[:]
            sr.ap = _vp([[32, 128], [4096, 2], [1, 32]])
            nc.sync.dma_start(S1c[:], sr)
            S1 = tpool.tile([128, 64], BF16)
            nc.scalar.activation(S1[:], S1c[:], AF.Relu)

            P2 = pse.tile([128, 512], F32, tag="pacc")
            for mc in range(16):
                msl = slice(32 * mc, 32 * mc + 32)
                for kc in range(2):
                    nc.tensor.matmul(P2[:, msl],
                                     fw2sb[:, 2048 * kc + 128 * mc:2048 * kc + 128 * mc + 128],
                                     S1[:, 32 * kc:32 * kc + 32],
                                     start=(kc == 0), stop=False)
                nc.tensor.matmul(P2[:, msl], fb2sb[:, 128 * mc:128 * mc + 128],
                                 ones1[:], start=False, stop=True)
            S2pre = tpool.tile([128, 512], BF16)
            nc.scalar.activation(S2pre[:], P2[:], AF.Copy)
            d2 = rs2i[:]
            d2.ap = _vp([[32, 128], [4096, 16], [1, 32]])
            nc.sync.dma_start(d2, S2pre[:])
            nc.gpsimd.collective_compute(
                "ReduceScatter", mybir.AluOpType.add, replica_groups=RG,
                ins=[rs2i.opt()], outs=[rs2o.opt()])
            S2c = tpool.tile([128, 64], BF16)
            sr2 = rs2o[:]
            sr2.ap = _vp([[32, 128], [4096, 2], [1, 32]])
            nc.sync.dma_start(S2c[:], sr2)
            S2 = tpool.tile([128, 64], BF16)
            nc.scalar.activation(S2[:], S2c[:], AF.Relu)

            P3 = pse2.tile([53, 32], F32, tag="ps5")
            for kc in range(2):
                nc.tensor.matmul(P3[:], fw3sb[:, 53 * kc:53 * kc + 53],
                                 S2[:, 32 * kc:32 * kc + 32],
                                 start=(kc == 0), stop=False)
            nc.tensor.matmul(P3[:], fb3sb[:], ones1[:], start=False, stop=True)
            S3 = tpool.tile([53, 32], F32)
            nc.scalar.activation(S3[:], P3[:], AF.Copy)
            nc.sync.dma_start(y_out[:].rearrange("b o -> o b"), S3[:])

    orig = nc.to_json_bytes
    nc.to_json_bytes = lambda: _fix_bir_json(orig())
    return nc


# ---------------------------------------------------------------- host prep
def _host_shared(inputs):
    f32 = np.float32
    cw = [np.asarray(inputs[f"cw{i+1}"], f32) for i in range(4)]
    cb = [np.asarray(inputs[f"cb{i+1}"], f32) for i in range(4)]
    d = {}
    t = np.zeros((108, 128), f32)
    for ky in range(3):
        for kx in range(3):
            blk = cw[0][:, :, ky, kx].T
            for img in range(4):
                r = (ky * 3 + kx) * 12 + img * 3
                t[r:r + 3, img * 32:(img + 1) * 32] = blk
    d["cw1p"] = t.astype(BF)
    d["cb1r"] = np.tile(cb[0], 4).reshape(128, 1)
    t = np.zeros((96, 192), f32)
    for ky in range(3):
        for kx in range(3):
            t[ky * 32:(ky + 1) * 32, kx * 64:(kx + 1) * 64] = cw[1][:, :, ky, kx].T
    d["cw2p"] = t.astype(BF)
    d["cb2r"] = cb[1].reshape(64, 1)
    ta = np.zeros((128, 384), f32)
    tb = np.zeros((64, 384), f32)
    for kx in range(3):
        for ky in range(2):
            ta[ky * 64:(ky + 1) * 64, kx * 128:(kx + 1) * 128] = cw[2][:, :, ky, kx].T
        tb[:, kx * 128:(kx + 1) * 128] = cw[2][:, :, 2, kx].T
    d["cw3ap"] = ta.astype(BF)
    d["cw3bp"] = tb.astype(BF)
    d["cb3r"] = cb[2].reshape(128, 1)
    t = np.zeros((128, 1152), f32)
    for k in range(9):
        ky, kx = divmod(k, 3)
        t[:, k * 128:(k + 1) * 128] = cw[3][:, :, ky, kx].T
    d["cw4p"] = t.astype(BF)
    d["cb4r"] = cb[3].reshape(128, 1)
    d["awr"] = np.asarray(inputs["aw"], f32)[:, :, 0].astype(BF)
    d["abr"] = np.asarray(inputs["ab"], f32).reshape(128, 1)
    return d


def _host_shard(inputs, c):
    f32 = np.float32
    E0 = EPC * c
    ew = [np.asarray(inputs[f"ew{i+1}"], f32)[E0:E0 + EPC] for i in range(5)]
    eb = [np.asarray(inputs[f"eb{i+1}"], f32)[E0:E0 + EPC] for i in range(5)]
    km = lambda a: np.ascontiguousarray(a.transpose(1, 0, 2))
    aug = lambda w, b: np.concatenate([w, b[None]], 0)
    d = {}
    for li, pre in ((0, "e1"), (1, "e2")):
        w, b = km(ew[li]), eb[li]
        d[pre + "AA"] = w[0:128, :, 0:128].reshape(128, -1).astype(BF)
        d[pre + "AB"] = w[0:128, :, 128:196].reshape(128, -1).astype(BF)
        d[pre + "BA"] = aug(w[128:196, :, 0:128], b[:, 0:128]).reshape(69, -1).astype(BF)
        d[pre + "BB"] = aug(w[128:196, :, 128:196], b[:, 128:196]).reshape(69, -1).astype(BF)
    w3 = km(ew[2])
    d["e3A"] = w3[0:128].reshape(128, -1).astype(BF)
    d["e3B"] = aug(w3[128:196], eb[2]).reshape(69, -1).astype(BF)
    d["e4"] = aug(km(ew[3]), eb[3]).reshape(99, -1).astype(BF)
    t = np.zeros((64, 256), f32)
    for p in range(8):
        for e2 in range(2):
            e = 2 * p + e2
            rb, cb_ = 32 * e2, p * 32 + e2 * 16
            t[rb:rb + 24, cb_:cb_ + 16] = ew[4][e]
            t[rb + 24, cb_:cb_ + 16] = eb[4][e]
    d["e5p"] = t.astype(BF)

    fw1 = np.asarray(inputs["fw1"], f32)
    t = np.zeros((2, 128, 2048), f32)
    for g in range(2):
        for row in range(128):
            pp, r32 = divmod(row, 32)
            e2, o = divmod(r32, 16)
            el = (g * 4 + pp) * 2 + e2
            t[g, row, 0:FIN[1]] = fw1[(E0 + el) * 16 + o]
    d["fw1p"] = t.astype(BF)
    d["fb1d8"] = np.pad(np.asarray(inputs["fb1"], f32) / 8,
                        (0, 2048 - FIN[1])).reshape(1, 2048).astype(BF)
    fw2 = np.asarray(inputs["fw2"], f32)
    t = np.zeros((2, 128, 2048), f32)
    for kc in range(2):
        m0 = 256 * c + kc * 128
        n = max(0, min(128, FIN[1] - m0))
        if n > 0:
            t[kc, :n, 0:FIN[2]] = fw2[m0:m0 + n]
    d["fw2p"] = t.astype(BF)
    d["fb2d8"] = np.pad(np.asarray(inputs["fb2"], f32) / 8,
                        (0, 2048 - FIN[2])).reshape(1, 2048).astype(BF)
    fw3 = np.asarray(inputs["fw3"], f32)
    t = np.zeros((2, 128, 53), f32)
    for kc in range(2):
        m0 = 256 * c + kc * 128
        n = max(0, min(128, FIN[2] - m0))
        if n > 0:
            t[kc, :n] = fw3[m0:m0 + n]
    d["fw3p"] = t.astype(BF)
    d["fb3d8"] = (np.asarray(inputs["fb3"], f32) / 8).reshape(1, 53).astype(BF)
    d["onesrow"] = np.ones((1, 512), BF)
    t5f = np.zeros((64, 256), BF)
    t5f[24, :] = 1
    t5f[56, :] = 1
    d["t5fill"] = t5f
    return d


def _in_maps(inputs):
    shared = _host_shared(inputs)
    x = np.asarray(inputs["x"], np.float32)
    maps = []
    for c in range(NCORES):
        m = dict(shared)
        m.update(_host_shard(inputs, c))
        xp = np.zeros((BL, 3, 226, 226), BF)
        xp[:, :, 1:225, 1:225] = x[c * BL:(c + 1) * BL]
        m["xpad"] = np.concatenate([xp.reshape(-1), np.zeros(2, BF)])
        maps.append(m)
    return maps


def kernel(**inputs):
    global _BUILT
    if _BUILT is None:
        _BUILT = _build()
    res = run_bass_kernel_spmd(_BUILT, _in_maps(inputs), list(range(NCORES)))
    return np.sum([res.results[c]["y"] for c in range(NCORES)], axis=0,
                  dtype=np.float32)


# revision 25
# speedup vs baseline: 2.9945x; 1.1792x over previous
"""TRN2 Bass kernel for nn_CardClassifier.

CNN(4x conv3x3+relu+maxpool2) -> per-feature sigmoid attention ->
128 stacked expert MLPs -> fusion MLP (2048->2038->2028->53).

Distribution: data-parallel convs (8 cores x 4 images), then AllToAll to
expert parallelism (16 experts/core x 32 images), K-sharded fusion with
two ReduceScatters; final 53-dim partials summed on the host.

Conv engines: tap-folded K packing (im2col replicas built by strided DMA
from a host-padded input / shifted SBUF-SBUF copies), bf16 weights and
activations, fp32 PSUM. Bias+relu folded after each maxpool (commute).
"""

import sys

sys.path.insert(0, "/opt/trn_rl_repo")

import json as _json
import contextlib
import numpy as np
import ml_dtypes

import bass_rust
import concourse.bass as bass
import concourse.mybir as mybir
from concourse import tile
from concourse.bass_utils import run_bass_kernel_spmd

F32 = mybir.dt.float32
BF16 = mybir.dt.bfloat16
AF = mybir.ActivationFunctionType
ALU = None  # filled lazily
BF = ml_dtypes.bfloat16

B, H, W = 32, 224, 224
NCORES, BL = 8, 4
NF, FLAT = 128, 196
EXP_DIMS = [196, 196, 196, 98, 24, 16]
FIN = [2048, 2038, 2028, 53]
EPC = 16  # experts per core

_BUILT = None
RG = [list(range(NCORES))]


# ---------------------------------------------------------------- tilefix
def _fix_bir_json(raw: bytes) -> bytes:
    """This walrus build allows at most 1 sync-wait per instruction; Tile's
    tail drain can carry more. Split extras onto NoOp carriers."""
    d = _json.loads(raw)
    k = 0
    for fn in d.get("functions", []):
        for blk in fn.get("blocks", []):
            out = []
            for inst in blk["instructions"]:
                si = inst.get("sync_info")
                waits = (si or {}).get("on_wait") or []
                if len(waits) > 1:
                    for wchunk in waits[:-1]:
                        out.append({
                            "debug": inst.get("debug", 0),
                            "engine": inst["engine"],
                            "ins": [], "outs": [],
                            "name": f"NOPW-{k}",
                            "opcode": "NoOp",
                            "sync_info": {"on_update": [], "on_wait": [wchunk]},
                        })
                        k += 1
                    si["on_wait"] = waits[-1:]
                out.append(inst)
            blk["instructions"] = out
    return _json.dumps(d).encode()


def _vp(dims):
    return bass_rust.VecI64Pair(dims)


# ---------------------------------------------------------------- build
def _build():
    global ALU
    from concourse.alu_op_type import AluOpType as ALU_

    ALU = ALU_
    nc = bass.Bass("TRN2", target_bir_lowering=False, debug=False,
                   num_devices=NCORES)

    dp = lambda name, shape, dt: nc.declare_dram_parameter(name, list(shape), dt, isOutput=False)

    xp = dp("xpad9", [108 * 226 * 226], BF16)  # host-materialized 9-tap im2col replicas
    cw1p = dp("cw1p", [108, 128], BF16)
    cb1r = dp("cb1r", [128, 1], F32)
    cw2p = dp("cw2p", [96, 192], BF16)
    cb2r = dp("cb2r", [64, 1], F32)
    cw3ap = dp("cw3ap", [128, 384], BF16)
    cw3bp = dp("cw3bp", [64, 384], BF16)
    cb3r = dp("cb3r", [128, 1], F32)
    cw4p = dp("cw4p", [128, 1152], BF16)
    cb4r = dp("cb4r", [128, 1], F32)
    awr = dp("awr", [128, FLAT], BF16)
    abr = dp("abr", [128, 1], F32)

    ew_shapes = {
        "e1AA": (128, EPC * 128), "e1AB": (128, EPC * 68),
        "e1BA": (69, EPC * 128), "e1BB": (69, EPC * 68),
        "e2AA": (128, EPC * 128), "e2AB": (128, EPC * 68),
        "e2BA": (69, EPC * 128), "e2BB": (69, EPC * 68),
        "e3A": (128, EPC * 98), "e3B": (69, EPC * 98),
        "e4": (99, EPC * 24),
        "e5p": (64, 256),
    }
    ew_in = {k: dp(k, list(s), BF16) for k, s in ew_shapes.items()}

    fw1_in = dp("fw1p", [2, 128, 2048], BF16)
    fb1_in = dp("fb1d8", [1, 2048], BF16)
    fw2_in = dp("fw2p", [2, 128, 2048], BF16)
    fb2_in = dp("fb2d8", [1, 2048], BF16)
    fw3_in = dp("fw3p", [2, 128, 53], BF16)
    fb3_in = dp("fb3d8", [1, 53], BF16)
    ones_in = dp("onesrow", [1, 512], BF16)
    t5f_in = dp("t5fill", [64, 256], BF16)

    y_out = nc.declare_dram_parameter("y", [32, 53], F32, isOutput=True)

    with tile.TileContext(nc, pool_alloc_mode="queue") as tc:
        stk = contextlib.ExitStack()
        with stk:
            # ---- persistent consts
            wpool = stk.enter_context(tc.tile_pool(name="wconst", bufs=1))
            cw1 = wpool.tile([108, 128], BF16)
            nc.sync.dma_start(cw1[:], cw1p[:])
            cw2 = wpool.tile([96, 192], BF16)
            nc.sync.dma_start(cw2[:], cw2p[:])
            cw3a = wpool.tile([128, 384], BF16)
            nc.sync.dma_start(cw3a[:], cw3ap[:])
            cw3b = wpool.tile([64, 384], BF16)
            nc.sync.dma_start(cw3b[:], cw3bp[:])
            cw4 = wpool.tile([128, 1152], BF16)
            nc.sync.dma_start(cw4[:], cw4p[:])
            cb1 = wpool.tile([128, 1], F32)
            nc.sync.dma_start(cb1[:], cb1r[:])
            cb2 = wpool.tile([64, 1], F32)
            nc.sync.dma_start(cb2[:], cb2r[:])
            cb3 = wpool.tile([128, 1], F32)
            nc.sync.dma_start(cb3[:], cb3r[:])
            cb4 = wpool.tile([128, 1], F32)
            nc.sync.dma_start(cb4[:], cb4r[:])
            awsb = wpool.tile([128, FLAT], BF16)
            nc.sync.dma_start(awsb[:], awr[:])
            absb = wpool.tile([128, 1], F32)
            nc.sync.dma_start(absb[:], abr[:])
            ones1 = wpool.tile([1, 32], BF16)
            nc.vector.memset(ones1[:], 1.0)

            hpool = stk.enter_context(tc.tile_pool(name="hp", bufs=1))
            Ht = [hpool.tile([128, FLAT], BF16, name=f"ht{i}") for i in range(BL)]
            HB = [hpool.tile([128, FLAT], BF16, name=f"hb{i}") for i in range(BL)]

            # conv2 replica tiles (pool opened before a1r for LIFO release)
            a2pool = stk.enter_context(tc.tile_pool(name="a2r", bufs=1))
            A2Ra = [a2pool.tile([128, 56 * 58], BF16, name=f"a2a{i}")
                    for i in range(BL)]
            A2Rb = [a2pool.tile([64, 56 * 58], BF16, name=f"a2b{i}")
                    for i in range(BL)]

            # =========================================================
            # conv1: 3->32, im2col K=108 (9 taps x 4img x 3ch), M=128
            # 14 strips of 16 output rows; pool+bias+relu -> pm
            # =========================================================
            a1stk = contextlib.ExitStack()
            a1pool = a1stk.enter_context(tc.tile_pool(name="a1r", bufs=1))
            A1R = [a1pool.tile([96, 112 * 114], BF16, name=f"a1r{i}")
                   for i in range(BL)]

            with tc.tile_pool(name="c1x", bufs=2) as xpool, \
                 tc.tile_pool(name="c1v", bufs=2) as vpool, \
                 tc.tile_pool(name="c1m", bufs=2) as mpool, \
                 tc.tile_pool(name="c1pm", bufs=1) as pmpool, \
                 tc.tile_pool(name="ps1", bufs=6, space="PSUM") as psc:
                pm = pmpool.tile([128, 112 * 114], BF16)
                pmv = pm.rearrange("p (r c) -> p r c", c=114)
                nc.vector.memset(pmv[:, :, 0], 0.0)
                nc.vector.memset(pmv[:, :, 113], 0.0)
                for s in range(14):
                    r0 = 16 * s
                    X9 = xpool.tile([108, 16 * 226], BF16, tag="x9")
                    src = xp[:]
                    src.ap = _vp([[51076, 108], [1, 3616]])
                    src.offset = src.offset + r0 * 226
                    (nc.sync if s % 2 == 0 else nc.scalar).dma_start(X9[:], src)
                    X9v = X9.rearrange("p (r c) -> p r c", c=226)
                    pvs = vpool.tile([128, 8 * 112], F32, tag="pvs")
                    pvsv = pvs.rearrange("p (r c) -> p r c", c=112)
                    for t in range(8):
                        P = psc.tile([128, 448], F32, tag="acc")
                        nc.tensor.matmul(P[:], cw1[:],
                                         X9v[:, 2 * t:2 * t + 2, 0:224],
                                         start=True, stop=True)
                        nc.vector.tensor_reduce(
                            pvsv[:, t:t + 1, :],
                            P.rearrange("p (r c t) -> p c r t", r=2, c=112, t=2),
                            axis=mybir.AxisListType.XY, op=ALU.max)
                    nc.vector.tensor_scalar(pmv[:, 8 * s:8 * s + 8, 1:113], pvsv[:],
                                            cb1[:, 0:1], 0.0,
                                            op0=ALU.add, op1=ALU.max)
                # replicate pm into per-image (ky,ch) layouts
                for i in range(BL):
                    av = A1R[i].rearrange("p (r c) -> p r c", c=114)
                    nc.vector.memset(av[0:32, 0, :], 0.0)
                    nc.vector.memset(av[64:96, 111, :], 0.0)
                    sp = pmv[32 * i:32 * i + 32, :, :]
                    nc.sync.dma_start(av[32:64, 0:112, :], sp)
                    nc.scalar.dma_start(av[0:32, 1:112, :], sp[:, 0:111, :])
                    nc.gpsimd.dma_start(av[64:96, 0:111, :], sp[:, 1:112, :])

            # =========================================================
            # conv2: 32->64 per img, K=96 (3ky x 32ch), 3 kx passes, M=64
            # =========================================================
            with tc.tile_pool(name="c2v", bufs=1) as v2pool, \
                 tc.tile_pool(name="c2m", bufs=1) as m2pool, \
                 tc.tile_pool(name="ps2", bufs=6, space="PSUM") as psc2:
                for i in range(BL):
                    av = A1R[i].rearrange("p (r c) -> p r c", c=114)
                    pvs2 = v2pool.tile([64, 56 * 56], F32, tag="pvs2")
                    p2v = pvs2.rearrange("p (r c) -> p r c", c=56)
                    for t in range(28):
                        P = psc2.tile([64, 448], F32, tag="acc2")
                        for kx in range(3):
                            nc.tensor.matmul(P[:], cw2[:, 64 * kx:64 * kx + 64],
                                             av[:, 4 * t:4 * t + 4, kx:kx + 112],
                                             start=(kx == 0), stop=(kx == 2))
                        nc.vector.tensor_reduce(
                            p2v[:, 2 * t:2 * t + 2, :],
                            P.rearrange("p (r a c b) -> p r c a b",
                                        r=2, a=2, c=56, b=2),
                            axis=mybir.AxisListType.XY, op=ALU.max)
                    pm2 = m2pool.tile([64, 56 * 58], BF16, tag="pm2")
                    pm2v = pm2.rearrange("p (r c) -> p r c", c=58)
                    nc.vector.memset(pm2v[:, :, 0], 0.0)
                    nc.vector.memset(pm2v[:, :, 57], 0.0)
                    nc.vector.tensor_scalar(pm2v[:, :, 1:57], pvs2[:],
                                            cb2[:, 0:1], 0.0,
                                            op0=ALU.add, op1=ALU.max)
                    aav = A2Ra[i].rearrange("p (r c) -> p r c", c=58)
                    abv = A2Rb[i].rearrange("p (r c) -> p r c", c=58)
                    nc.vector.memset(aav[0:64, 0, :], 0.0)
                    nc.vector.memset(abv[:, 55, :], 0.0)
                    nc.sync.dma_start(aav[64:128, 0:56, :], pm2v[:])
                    nc.scalar.dma_start(aav[0:64, 1:56, :], pm2v[:, 0:55, :])
                    nc.gpsimd.dma_start(abv[:, 0:55, :], pm2v[:, 1:56, :])
            a1stk.close()

            # ---- expert + fusion weights to SBUF (during conv3/4;
            # pool opened after A1R frees so the ring has room)
            ewfpool = stk.enter_context(tc.tile_pool(name="ewf", bufs=1))
            ewsb = {}
            for k, s in ew_shapes.items():
                t = ewfpool.tile(list(s), BF16, name=k + "sb")
                nc.gpsimd.dma_start(t[:], ew_in[k][:])
                ewsb[k] = t
            fw1sb = ewfpool.tile([128, 4096], BF16)
            s1 = fw1_in[:]
            s1.ap = _vp([[2048, 128], [262144, 2], [1, 2048]])
            nc.gpsimd.dma_start(fw1sb[:], s1)
            fw2sb = ewfpool.tile([128, 4096], BF16)
            s2 = fw2_in[:]
            s2.ap = _vp([[2048, 128], [262144, 2], [1, 2048]])
            nc.gpsimd.dma_start(fw2sb[:], s2)
            fw3sb = ewfpool.tile([128, 106], BF16)
            s3 = fw3_in[:]
            s3.ap = _vp([[53, 128], [6784, 2], [1, 53]])
            nc.gpsimd.dma_start(fw3sb[:], s3)
            fb1sb = ewfpool.tile([1, 2048], BF16)
            nc.gpsimd.dma_start(fb1sb[:], fb1_in[:])
            fb2sb = ewfpool.tile([1, 2048], BF16)
            nc.gpsimd.dma_start(fb2sb[:], fb2_in[:])
            fb3sb = ewfpool.tile([1, 53], BF16)
            nc.gpsimd.dma_start(fb3sb[:], fb3_in[:])

            # =========================================================
            # conv3: 64->128 per img, K=128 (2ky x 64ch) + K=64 (ky2)
            # =========================================================
            a3pool = stk.enter_context(tc.tile_pool(name="a3m", bufs=1))
            A3 = [a3pool.tile([128, 30 * 30], BF16, name=f"a3_{i}")
                  for i in range(BL)]

            with tc.tile_pool(name="c3v", bufs=2) as v3pool, \
                 tc.tile_pool(name="ps3", bufs=6, space="PSUM") as psc3:
                aavs = [A2Ra[i].rearrange("p (r c) -> p r c", c=58) for i in range(BL)]
                abvs = [A2Rb[i].rearrange("p (r c) -> p r c", c=58) for i in range(BL)]
                pvs3s = [v3pool.tile([128, 28 * 28], F32, tag=f"pvs3_{i}", name=f"pvs3_{i}")
                         for i in range(BL)]
                p3vs = [t.rearrange("p (r c) -> p r c", c=28) for t in pvs3s]
                for g2 in range(2):
                    imgs = (2 * g2, 2 * g2 + 1)
                    for t in range(7):
                        Ps = {i: psc3.tile([128, 448], F32, tag="acc3", name=f"p3t{g2}_{t}_{i}")
                              for i in imgs}
                        for kx in range(3):
                            for i in imgs:
                                nc.tensor.matmul(Ps[i][:], cw3a[:, 128 * kx:128 * kx + 128],
                                                 aavs[i][:, 8 * t:8 * t + 8, kx:kx + 56],
                                                 start=(kx == 0), stop=False)
                            for i in imgs:
                                nc.tensor.matmul(Ps[i][:], cw3b[:, 128 * kx:128 * kx + 128],
                                                 abvs[i][:, 8 * t:8 * t + 8, kx:kx + 56],
                                                 start=False, stop=(kx == 2))
                        for i in imgs:
                            nc.vector.tensor_reduce(
                                p3vs[i][:, 4 * t:4 * t + 4, :],
                                Ps[i].rearrange("p (r a c b) -> p r c a b",
                                                r=4, a=2, c=28, b=2),
                                axis=mybir.AxisListType.XY, op=ALU.max)
                    for i in imgs:
                        a3v = A3[i].rearrange("p (r c) -> p r c", c=30)
                        nc.vector.memset(a3v[:, 0, :], 0.0)
                        nc.vector.memset(a3v[:, 29, :], 0.0)
                        nc.vector.memset(a3v[:, :, 0], 0.0)
                        nc.vector.memset(a3v[:, :, 29], 0.0)
                        nc.vector.tensor_scalar(a3v[:, 1:29, 1:29], pvs3s[i][:],
                                                cb3[:, 0:1], 0.0,
                                                op0=ALU.add, op1=ALU.max)

                # =====================================================
                # conv4: 128->128 per img, K=128, 9 taps via views
                # =====================================================
                a3vs = [A3[i].rearrange("p (r c) -> p r c", c=30) for i in range(BL)]
                pvs4s = [v3pool.tile([128, 14 * 14], F32, tag=f"pvs4_{i}", name=f"pvs4_{i}")
                         for i in range(BL)]
                p4vs = [t.rearrange("p (r c) -> p r c", c=14) for t in pvs4s]
                for g2 in range(2):
                    imgs = (2 * g2, 2 * g2 + 1)
                    for t, (rb, nr) in enumerate(((0, 16), (16, 12))):
                        Ps = {i: psc3.tile([128, nr * 28], F32, tag="acc3",
                                           name=f"p4t{g2}_{t}_{i}") for i in imgs}
                        for k in range(9):
                            ky, kx = divmod(k, 3)
                            for i in imgs:
                                nc.tensor.matmul(Ps[i][:], cw4[:, 128 * k:128 * k + 128],
                                                 a3vs[i][:, rb + ky:rb + ky + nr, kx:kx + 28],
                                                 start=(k == 0), stop=(k == 8))
                        for i in imgs:
                            nc.vector.tensor_reduce(
                                p4vs[i][:, rb // 2:rb // 2 + nr // 2, :],
                                Ps[i].rearrange("p (r a c b) -> p r c a b",
                                                r=nr // 2, a=2, c=14, b=2),
                                axis=mybir.AxisListType.XY, op=ALU.max)
                    for i in imgs:
                        nc.vector.tensor_scalar(Ht[i][:], pvs4s[i][:],
                                                cb4[:, 0:1], 0.0,
                                                op0=ALU.add, op1=ALU.max)

            # =========================================================
            # attention: att = sigmoid(feats . aw + ab); h = feats * att
            # =========================================================
            with tc.tile_pool(name="att", bufs=2) as atp:
                for i in range(BL):
                    tmp = atp.tile([128, FLAT], F32, tag="tmp")
                    nc.vector.tensor_tensor(tmp[:], Ht[i][:], awsb[:], op=ALU.mult)
                    attv = atp.tile([128, 1], F32, tag="av")
                    nc.vector.tensor_reduce(attv[:], tmp[:],
                                            axis=mybir.AxisListType.X, op=ALU.add)
                    atts = atp.tile([128, 1], F32, tag="as")
                    nc.scalar.activation(atts[:], attv[:], AF.Sigmoid, bias=absb[:])
                    nc.vector.tensor_scalar(HB[i][:], Ht[i][:], atts[:, 0:1],
                                            None, op0=ALU.mult)

            # =========================================================
            # AllToAll: [128e, 4i x 256fpad] -> [8s x 16e, 4i x 256fpad]
            # =========================================================
            dram = stk.enter_context(tc.tile_pool(name="dram", bufs=1, space="DRAM"))
            in_b = dram.tile([128, 1024], BF16)
            out_b = dram.tile([128, 1024], BF16)
            ibv = in_b.rearrange("p (i f) -> p i f", f=256)
            for i in range(BL):
                nc.sync.dma_start(ibv[:, i, 0:FLAT], HB[i][:])
            nc.gpsimd.collective_compute(
                "AllToAll", mybir.AluOpType.bypass, replica_groups=RG,
                ins=[in_b.opt()], outs=[out_b.opt()])

            # transpose to [f, (s,e,i)] then relabel cols to (e, g=4s+i)
            tpool = stk.enter_context(tc.tile_pool(name="texp", bufs=1))
            TAraw = tpool.tile([128, 512], BF16)
            TBraw = tpool.tile([128, 512], BF16)
            for blk, dst in ((0, TAraw), (1, TBraw)):
                src = out_b[:]
                src.ap = _vp([[256, 512], [1, 128]])
                src.offset = src.offset + 128 * blk
                nc.sync.dma_start_transpose(dst[:], src)
            TA1 = tpool.tile([128, 512], BF16)
            TB1 = tpool.tile([69, 512], BF16)
            TAv = TA1.rearrange("p (e g) -> p e g", g=32)
            TBv = TB1.rearrange("p (e g) -> p e g", g=32)
            for s in range(8):
                nc.vector.tensor_copy(
                    TAv[:, :, 4 * s:4 * s + 4],
                    TAraw[:, 64 * s:64 * s + 64].rearrange("p (e i) -> p e i", i=4))
                nc.vector.tensor_copy(
                    TBv[0:68, :, 4 * s:4 * s + 4],
                    TBraw[0:68, 64 * s:64 * s + 64].rearrange("p (e i) -> p e i", i=4))
            nc.sync.dma_start(TB1[68:69, :], ones_in[:])

            # =========================================================
            # experts: 16 local experts x 32 imgs, weight-stationary
            # =========================================================
            pse = stk.enter_context(tc.tile_pool(name="pse", bufs=5, space="PSUM"))
            pse2 = stk.enter_context(tc.tile_pool(name="pse2", bufs=2, space="PSUM"))

            def elayer(TAi, TBi, pre):
                PA = pse.tile([128, 512], F32, tag="pacc")
                PB = pse.tile([68, 512], F32, tag="pacc")
                wAA, wAB = ewsb[pre + "AA"], ewsb[pre + "AB"]
                wBA, wBB = ewsb[pre + "BA"], ewsb[pre + "BB"]
                for e in range(EPC):
                    sl = slice(32 * e, 32 * e + 32)
                    nc.tensor.matmul(PA[:, sl], wAA[:, 128 * e:128 * e + 128],
                                     TAi[:, sl], start=True, stop=False)
                    nc.tensor.matmul(PA[:, sl], wBA[:, 128 * e:128 * e + 128],
                                     TBi[:, sl], start=False, stop=True)
                    nc.tensor.matmul(PB[:, sl], wAB[:, 68 * e:68 * e + 68],
                                     TAi[:, sl], start=True, stop=False)
                    nc.tensor.matmul(PB[:, sl], wBB[:, 68 * e:68 * e + 68],
                                     TBi[:, sl], start=False, stop=True)
                TAo = tpool.tile([128, 512], BF16, name=pre + "oa")
                TBo = tpool.tile([69, 512], BF16, name=pre + "ob")
                nc.scalar.activation(TAo[:], PA[:], AF.Relu)
                nc.scalar.activation(TBo[0:68, :], PB[:], AF.Relu)
                nc.sync.dma_start(TBo[68:69, :], ones_in[:])
                return TAo, TBo

            TA2, TB2 = elayer(TA1, TB1, "e1")
            TA3, TB3 = elayer(TA2, TB2, "e2")

            P98 = pse.tile([98, 512], F32, tag="pacc")
            for e in range(EPC):
                sl = slice(32 * e, 32 * e + 32)
                nc.tensor.matmul(P98[:, sl], ewsb["e3A"][:, 98 * e:98 * e + 98],
                                 TA3[:, sl], start=True, stop=False)
                nc.tensor.matmul(P98[:, sl], ewsb["e3B"][:, 98 * e:98 * e + 98],
                                 TB3[:, sl], start=False, stop=True)
            T4 = tpool.tile([99, 512], BF16)
            nc.scalar.activation(T4[0:98, :], P98[:], AF.Relu)
            nc.sync.dma_start(T4[98:99, :], ones_in[:])

            P24 = pse.tile([24, 512], F32, tag="pacc")
            for e in range(EPC):
                sl = slice(32 * e, 32 * e + 32)
                nc.tensor.matmul(P24[:, sl], ewsb["e4"][:, 24 * e:24 * e + 24],
                                 T4[:, sl], start=True, stop=True)

            # pair layout for L5: rows 0..24 even expert, 32..56 odd
            # (zeros + ones-rows prefilled from DRAM, data rows overwritten)
            T5R = tpool.tile([64, 256], BF16)
            nc.sync.dma_start(T5R[:], t5f_in[:])
            P24v = P24.rearrange("p (e g) -> p e g", g=32)
            T5Rv = T5R.rearrange("p (q g) -> p q g", g=32)
            nc.scalar.activation(T5Rv[0:24, :, :], P24v[0:24, 0:16:2, :], AF.Relu)
            nc.scalar.activation(T5Rv[32:56, :, :], P24v[0:24, 1:16:2, :], AF.Relu)

            S5 = pse2.tile([128, 64], F32, tag="ps5")
            for p in range(8):
                nc.tensor.matmul(S5[32 * (p % 4):32 * (p % 4) + 32,
                                    32 * (p // 4):32 * (p // 4) + 32],
                                 ewsb["e5p"][:, 32 * p:32 * p + 32],
                                 T5R[:, 32 * p:32 * p + 32],
                                 start=True, stop=True,
                                 tile_position=(0, 32 * (p % 4)))
            SF = tpool.tile([128, 64], BF16)
            nc.scalar.activation(SF[:], S5[:], AF.Relu)

            # =========================================================
            # fusion: K-sharded partials + ReduceScatter x2, host sum
            # =========================================================
            rs1i = dram.tile([2048, 32], BF16)
            rs1o = dram.tile([256, 32], BF16)
            rs2i = dram.tile([2048, 32], BF16)
            rs2o = dram.tile([256, 32], BF16)

            P1 = pse.tile([128, 512], F32, tag="pacc")
            for mc in range(16):
                msl = slice(32 * mc, 32 * mc + 32)
                for g in range(2):
                    nc.tensor.matmul(P1[:, msl],
                                     fw1sb[:, 2048 * g + 128 * mc:2048 * g + 128 * mc + 128],
                                     SF[:, 32 * g:32 * g + 32],
                                     start=(g == 0), stop=False)
                nc.tensor.matmul(P1[:, msl], fb1sb[:, 128 * mc:128 * mc + 128],
                                 ones1[:], start=False, stop=True)
            S1pre = tpool.tile([128, 512], BF16)
            nc.scalar.activation(S1pre[:], P1[:], AF.Copy)
            d1 = rs1i[:]
            d1.ap = _vp([[32, 128], [4096, 16], [1, 32]])
            nc.sync.dma_start(d1, S1pre[:])
            nc.gpsimd.collective_compute(
                "ReduceScatter", mybir.AluOpType.add, replica_groups=RG,
                ins=[rs1i.opt()], outs=[rs1o.opt()])
            S1c = tpool.tile([128, 64], BF16)
            sr = rs1o[:]
            sr.ap = _vp([[32, 128], [4096, 2], [1, 32]])
            nc.sync.dma_start(S1c[:], sr)
            S1 = tpool.tile([128, 64], BF16)
            nc.scalar.activation(S1[:], S1c[:], AF.Relu)

            P2 = pse.tile([128, 512], F32, tag="pacc")
            for mc in range(16):
                msl = slice(32 * mc, 32 * mc + 32)
                for kc in range(2):
                    nc.tensor.matmul(P2[:, msl],
                                     fw2sb[:, 2048 * kc + 128 * mc:2048 * kc + 128 * mc + 128],
                                     S1[:, 32 * kc:32 * kc + 32],
                                     start=(kc == 0), stop=False)
                nc.tensor.matmul(P2[:, msl], fb2sb[:, 128 * mc:128 * mc + 128],
                                 ones1[:], start=False, stop=True)
            S2pre = tpool.tile([128, 512], BF16)
            nc.scalar.activation(S2pre[:], P2[:], AF.Copy)
            d2 = rs2i[:]
            d2.ap = _vp([[32, 128], [4096, 16], [1, 32]])
            nc.sync.dma_start(d2, S2pre[:])
            nc.gpsimd.collective_compute(
                "ReduceScatter", mybir.AluOpType.add, replica_groups=RG,
                ins=[rs2i.opt()], outs=[rs2o.opt()])
            S2c = tpool.tile([128, 64], BF16)
            sr2 = rs2o[:]
            sr2.ap = _vp([[32, 128], [4096, 2], [1, 32]])
            nc.sync.dma_start(S2c[:], sr2)
            S2 = tpool.tile([128, 64], BF16)
            nc.scalar.activation(S2[:], S2c[:], AF.Relu)

            P3 = pse2.tile([53, 32], F32, tag="ps5")
            for kc in range(2):
                nc.tensor.matmul(P3[:], fw3sb[:, 53 * kc:53 * kc + 53],
                                 S2[:, 32 * kc:32 * kc + 32],
                                 start=(kc == 0), stop=False)
            nc.tensor.matmul(P3[:], fb3sb[:], ones1[:], start=False, stop=True)
            S3 = tpool.tile([53, 32], F32)
            nc.scalar.activation(S3[:], P3[:], AF.Copy)
            nc.sync.dma_start(y_out[:].rearrange("b o -> o b"), S3[:])

    orig = nc.to_json_bytes
    nc.to_json_bytes = lambda: _fix_bir_json(orig())
    return nc


# ---------------------------------------------------------------- host prep
def _host_shared(inputs):
    f32 = np.float32
    cw = [np.asarray(inputs[f"cw{i+1}"], f32) for i in range(4)]
    cb = [np.asarray(inputs[f"cb{i+1}"], f32) for i in range(4)]
    d = {}
    t = np.zeros((108, 128), f32)
    for ky in range(3):
        for kx in range(3):
            blk = cw[0][:, :, ky, kx].T
            for img in range(4):
                r = (ky * 3 + kx) * 12 + img * 3
                t[r:r + 3, img * 32:(img + 1) * 32] = blk
    d["cw1p"] = t.astype(BF)
    d["cb1r"] = np.tile(cb[0], 4).reshape(128, 1)
    t = np.zeros((96, 192), f32)
    for ky in range(3):
        for kx in range(3):
            t[ky * 32:(ky + 1) * 32, kx * 64:(kx + 1) * 64] = cw[1][:, :, ky, kx].T
    d["cw2p"] = t.astype(BF)
    d["cb2r"] = cb[1].reshape(64, 1)
    ta = np.zeros((128, 384), f32)
    tb = np.zeros((64, 384), f32)
    for kx in range(3):
        for ky in range(2):
            ta[ky * 64:(ky + 1) * 64, kx * 128:(kx + 1) * 128] = cw[2][:, :, ky, kx].T
        tb[:, kx * 128:(kx + 1) * 128] = cw[2][:, :, 2, kx].T
    d["cw3ap"] = ta.astype(BF)
    d["cw3bp"] = tb.astype(BF)
    d["cb3r"] = cb[2].reshape(128, 1)
    t = np.zeros((128, 1152), f32)
    for k in range(9):
        ky, kx = divmod(k, 3)
        t[:, k * 128:(k + 1) * 128] = cw[3][:, :, ky, kx].T
    d["cw4p"] = t.astype(BF)
    d["cb4r"] = cb[3].reshape(128, 1)
    d["awr"] = np.asarray(inputs["aw"], f32)[:, :, 0].astype(BF)
    d["abr"] = np.asarray(inputs["ab"], f32).reshape(128, 1)
    return d


def _host_shard(inputs, c):
    f32 = np.float32
    E0 = EPC * c
    ew = [np.asarray(inputs[f"ew{i+1}"], f32)[E0:E0 + EPC] for i in range(5)]
    eb = [np.asarray(inputs[f"eb{i+1}"], f32)[E0:E0 + EPC] for i in range(5)]
    km = lambda a: np.ascontiguousarray(a.transpose(1, 0, 2))
    aug = lambda w, b: np.concatenate([w, b[None]], 0)
    d = {}
    for li, pre in ((0, "e1"), (1, "e2")):
        w, b = km(ew[li]), eb[li]
        d[pre + "AA"] = w[0:128, :, 0:128].reshape(128, -1).astype(BF)
        d[pre + "AB"] = w[0:128, :, 128:196].reshape(128, -1).astype(BF)
        d[pre + "BA"] = aug(w[128:196, :, 0:128], b[:, 0:128]).reshape(69, -1).astype(BF)
        d[pre + "BB"] = aug(w[128:196, :, 128:196], b[:, 128:196]).reshape(69, -1).astype(BF)
    w3 = km(ew[2])
    d["e3A"] = w3[0:128].reshape(128, -1).astype(BF)
    d["e3B"] = aug(w3[128:196], eb[2]).reshape(69, -1).astype(BF)
    d["e4"] = aug(km(ew[3]), eb[3]).reshape(99, -1).astype(BF)
    t = np.zeros((64, 256), f32)
    for p in range(8):
        for e2 in range(2):
            e = 2 * p + e2
            rb, cb_ = 32 * e2, p * 32 + e2 * 16
            t[rb:rb + 24, cb_:cb_ + 16] = ew[4][e]
            t[rb + 24, cb_:cb_ + 16] = eb[4][e]
    d["e5p"] = t.astype(BF)

    fw1 = np.asarray(inputs["fw1"], f32)
    t = np.zeros((2, 128, 2048), f32)
    for g in range(2):
        for row in range(128):
            pp, r32 = divmod(row, 32)
            e2, o = divmod(r32, 16)
            el = (g * 4 + pp) * 2 + e2
            t[g, row, 0:FIN[1]] = fw1[(E0 + el) * 16 + o]
    d["fw1p"] = t.astype(BF)
    d["fb1d8"] = np.pad(np.asarray(inputs["fb1"], f32) / 8,
                        (0, 2048 - FIN[1])).reshape(1, 2048).astype(BF)
    fw2 = np.asarray(inputs["fw2"], f32)
    t = np.zeros((2, 128, 2048), f32)
    for kc in range(2):
        m0 = 256 * c + kc * 128
        n = max(0, min(128, FIN[1] - m0))
        if n > 0:
            t[kc, :n, 0:FIN[2]] = fw2[m0:m0 + n]
    d["fw2p"] = t.astype(BF)
    d["fb2d8"] = np.pad(np.asarray(inputs["fb2"], f32) / 8,
                        (0, 2048 - FIN[2])).reshape(1, 2048).astype(BF)
    fw3 = np.asarray(inputs["fw3"], f32)
    t = np.zeros((2, 128, 53), f32)
    for kc in range(2):
        m0 = 256 * c + kc * 128
        n = max(0, min(128, FIN[2] - m0))
        if n > 0:
            t[kc, :n] = fw3[m0:m0 + n]
    d["fw3p"] = t.astype(BF)
    d["fb3d8"] = (np.asarray(inputs["fb3"], f32) / 8).reshape(1, 53).astype(BF)
    d["onesrow"] = np.ones((1, 512), BF)
    t5f = np.zeros((64, 256), BF)
    t5f[24, :] = 1
    t5f[56, :] = 1
    d["t5fill"] = t5f
    return d


def _in_maps(inputs):
    shared = _host_shared(inputs)
    x = np.asarray(inputs["x"], np.float32)
    maps = []
    for c in range(NCORES):
        m = dict(shared)
        m.update(_host_shard(inputs, c))
        xpl = np.zeros((BL, 3, 228, 228), np.float32)
        xpl[:, :, 1:225, 1:225] = x[c * BL:(c + 1) * BL]
        x9 = np.empty((9, 12, 226, 226), np.float32)
        for ky in range(3):
            for kx in range(3):
                x9[ky * 3 + kx] = xpl[:, :, ky:ky + 226, kx:kx + 226].reshape(12, 226, 226)
        m["xpad9"] = x9.reshape(-1).astype(BF)
        maps.append(m)
    return maps


def kernel(**inputs):
    global _BUILT
    if _BUILT is None:
        _BUILT = _build()
    res = run_bass_kernel_spmd(_BUILT, _in_maps(inputs), list(range(NCORES)))
    return np.sum([res.results[c]["y"] for c in range(NCORES)], axis=0,
                  dtype=np.float32)
